# revision 48
# baseline (speedup 1.0000x reference)
# MoE EnhancedGatedFusion kernel for 8x TRN2 NeuronCores (expert-parallel).
#
# Decomposition:
#   host : router logits -> top2 -> softmax gates -> dispatch by expert
#   L1   : per-core (expert e): H_T[d_out, n] = silu(We[e].T-contract @ XT + be[e])
#          (ungated; bf16 matmul operands, fp32 PSUM + fp32 H output)
#   host : combine CT = g1*A + g2*B during the token un-shuffle (the
#          "all-to-all return" glue), downcast once to bf16.
#   L2   : per-core (1024 tokens): out = CT.T @ Wo; y = XIN + out (XIN =
#          x_shard + bo folded on host, fp32); RMS-norm * norm_w.
#
# Matmul operands are bf16: same 1 cycle/row PE rate as fp32r but half the
# HBM/SBUF traffic, and FWL (fast weight load) kicks in for non-fp32 dtypes
# so LDWEIGHTS hides under the 64-deep PE reorder window.
import sys
import types

sys.path.insert(0, "/opt/trn_rl_repo")

import numpy as np
import ml_dtypes

BF16 = np.dtype(ml_dtypes.bfloat16)


def _install_ntff_hook():
    # antenv.axon_hooks is missing in this image; shim it so
    # run_bass_kernel_spmd(trace=True) can drive NTFF profiling.
    if "antenv.axon_hooks" in sys.modules:
        return
    try:
        from trn_agent_boot.trn_boot import _ntff_profile_via_ctypes

        hook = _ntff_profile_via_ctypes("/opt/axon/libaxon_pjrt.so")
    except Exception:
        hook = None
    mod = types.ModuleType("antenv.axon_hooks")
    mod.get_axon_ntff_profile_hook = lambda: hook
    mod.set_axon_ntff_profile_hook = lambda h: None
    sys.modules["antenv.axon_hooks"] = mod


_install_ntff_hook()

import concourse.bacc as bacc
import concourse.bass as bass
import concourse.tile as tile
from concourse import mybir
from concourse.bass_utils import run_bass_kernel_spmd

F32 = mybir.dt.float32
BF = mybir.dt.bfloat16
P = 128
NCORE = 8


def _chunks(total, size):
    out = []
    o = 0
    while o < total:
        out.append((o, min(size, total - o)))
        o += size
    return out


def build_l1(D, Bcap):
    """Per-core expert FFN: H[d_out, n] = silu(sum_k W[k,d_out]*XT[k,n] + be[d_out]).

    XT_T is host-pretiled bf16 [C, P, K, 512] (zero-padded ragged tail) and
    W_T is bf16 [F, P, K, P]; W is fully SBUF-resident (8.4MB), XT streams
    through once.  H output is fp32.

    The first 512-slab is processed as two 256-col sub-chunks so the
    lead-in only gates on 1MB of XT + the first W tile; remaining W tiles
    stream behind while the f-loop burns through the small chunk.
    """
    K = D // P          # k-tiles
    F = D // P          # feat (d_out) tiles
    chunks = [(0, 384)] + [(384 + o, n) for o, n in _chunks(Bcap - 384, 512)]
    Crest = len(chunks) - 1
    nc = bacc.Bacc("TRN2", target_bir_lowering=False, debug=False)
    XT0 = nc.dram_tensor("XT0", [P, K, 384], BF, kind="ExternalInput")
    XT = nc.dram_tensor("XT", [Crest, P, K, 512], BF, kind="ExternalInput")
    W = nc.dram_tensor("W", [F, P, K, P], BF, kind="ExternalInput")
    BE = nc.dram_tensor("BE", [D], F32, kind="ExternalInput")
    H = nc.dram_tensor("H", [D, Bcap], F32, kind="ExternalOutput")

    Hr = H[:, :].rearrange("(fo p) n -> p fo n", p=P)

    with tile.TileContext(nc) as tc:
        with (
            tc.tile_pool(name="consts", bufs=1) as consts,
            tc.tile_pool(name="xt", bufs=3) as xtp,
            tc.tile_pool(name="wf", bufs=1) as wfp,
            tc.tile_pool(name="hout", bufs=4) as hp,
            tc.tile_pool(name="ps", bufs=4, space="PSUM") as psp,
        ):
            # DMA issue order tuned for the lead-in: w0, be, first 256 cols
            # of XT, then remaining W tiles (second sub-chunk's XT slotted
            # midway so it lands before the f-loop reaches it).
            # PE warm-up: ~48 tiny matmuls during the DMA lead-in keep the
            # HAM activity window busy so real matmuls start at 2.4GHz.
            warm = consts.tile([P, 64], BF)
            nc.vector.memset(warm[:], 1.0)
            wps = psp.tile([P, 512], F32, tag="ps", name="ps")
            for _ in range(48):
                nc.tensor.matmul(wps[0:64, 0:64], lhsT=warm[:, 0:64],
                                 rhs=warm[:], start=True, stop=True)

            w_tiles = [None] * F
            w_tiles[0] = wfp.tile([P, K, P], BF, tag="wf0", name="wf0")
            nc.sync.dma_start(w_tiles[0][:, 0 : K // 2, :], W[0, :, 0 : K // 2, :])
            xt_tiles = {}
            xt_tiles[0] = xtp.tile([P, K, 512], BF, tag="xt", name="xt")
            nc.sync.dma_start(xt_tiles[0][:, 0 : K // 2, 0:384],
                              XT0[:, 0 : K // 2, :])
            nc.sync.dma_start(w_tiles[0][:, K // 2 : K, :], W[0, :, K // 2 : K, :])
            nc.sync.dma_start(xt_tiles[0][:, K // 2 : K, 0:384],
                              XT0[:, K // 2 : K, :])
            be_sb = consts.tile([P, F], F32)
            nc.sync.dma_start(be_sb[:], BE[:].rearrange("(f p) -> p f", p=P))
            for f in range(1, F):
                w_f = wfp.tile([P, K, P], BF, tag=f"wf{f}", name=f"wf{f}")
                nc.sync.dma_start(w_f[:], W[f])
                w_tiles[f] = w_f
                if f == 8:
                    # chunk 1's XT must land before chunk 0's (shortened,
                    # 384-col) f-loop drains — ahead of the last W tiles
                    xt_tiles[1] = xtp.tile([P, K, 512], BF, tag="xt", name="xt")
                    nc.sync.dma_start(xt_tiles[1][:], XT[0])

            for ci, (c0, cn) in enumerate(chunks):
                if ci in xt_tiles:
                    xt_c = xt_tiles[ci]
                else:
                    xt_c = xtp.tile([P, K, 512], BF, tag="xt", name="xt")
                    nc.sync.dma_start(xt_c[:, :, :cn],
                                      XT[ci - 1, :, :, :cn])
                for f in range(F):
                    ps = psp.tile([P, 512], F32, tag="ps", name="ps")
                    for k in range(K):
                        nc.tensor.matmul(
                            ps[:, :cn],
                            lhsT=w_tiles[f][:, k, :],
                            rhs=xt_c[:, k, :cn],
                            start=(k == 0),
                            stop=(k == K - 1),
                        )
                    h_t = hp.tile([P, 512], F32, tag="h", name="h")
                    nc.scalar.activation(
                        h_t[:, :cn],
                        ps[:, :cn],
                        mybir.ActivationFunctionType.Silu,
                        bias=be_sb[:, f : f + 1],
                        scale=1.0,
                    )
                    nc.sync.dma_start(Hr[:, f, c0 : c0 + cn], h_t[:, :cn])
    nc.compile()
    return nc


def build_l2(D, TPC, unit_nw, eps=1e-6):
    """Per-core combine + output proj + residual + RMS norm.

    Y[t, j] = nw[j] * (XIN[t,j] + sum_k CT[k,t]*Wo[k,j]) / rms(t)
    CT = g1*A + g2*B (host-combined, bf16); XIN = x_shard + bo (fp32).
    Y output is bf16 (host upcasts).

    The last n-chunk runs m-outer so each m's epilogue (RMS + scale +
    store) chains behind its own k-loop and overlaps the next m's
    matmuls; only m=M-1's epilogue trails the final matmul.

    unit_nw=True specializes norm_w == 1 (scale-by-rstd runs as a scalar
    engine activation; multiplying by 1 is exact) so the vector engine
    stays under the per-m tensor budget during the epilogue phase.
    """
    K = D // P
    M = TPC // P
    NC4 = D // 512
    KB = K // 4          # k-tiles bundled per DMA
    nc = bacc.Bacc("TRN2", target_bir_lowering=False, debug=False)
    # CT/WO are host-pretiled so every bundle DMA reads long contiguous
    # per-partition runs (8KB / 4KB) — strided reads from the natural
    # [D, x] layout only sustain ~40% of DMA bandwidth and gate the lead-in.
    CT = nc.dram_tensor("CT", [KB, P, 4, TPC], BF, kind="ExternalInput")
    XIN = nc.dram_tensor("XIN", [TPC, D], F32, kind="ExternalInput")
    WO = nc.dram_tensor("WO", [KB, NC4, P, 4, 512], BF, kind="ExternalInput")
    NW = nc.dram_tensor("NW", [D], F32, kind="ExternalInput")
    Y = nc.dram_tensor("Y", [TPC, D], BF, kind="ExternalOutput")

    XINr = XIN[:, :].rearrange("(m p) d -> p m d", p=P)

    with tile.TileContext(nc) as tc:
        with (
            tc.tile_pool(name="consts", bufs=1) as consts,
            tc.tile_pool(name="ct", bufs=1) as ctp,
            tc.tile_pool(name="wo", bufs=3) as wop,
            tc.tile_pool(name="yall", bufs=1) as yallp,
            tc.tile_pool(name="sq", bufs=3) as sqp,
            tc.tile_pool(name="yn", bufs=2) as ynp,
            tc.tile_pool(name="ssm", bufs=1) as ssmp,
            tc.tile_pool(name="stat", bufs=4) as statp,
            tc.tile_pool(name="ps", bufs=1, space="PSUM") as psp,
        ):
            # Bundled DMAs (4 k-tiles each), interleaved (wo, ct) pairs so
            # the k-loop's operands arrive in consumption order; XIN and nw
            # are queued behind everything n=0/n=1 needs.
            ct_bs = []
            wo_cur = []
            for b in range(KB):
                w_b = wop.tile([P, 4, 512], BF, tag=f"wo{b}", name=f"wo{b}")
                nc.sync.dma_start(w_b[:], WO[b, 0])
                wo_cur.append(w_b)
                c_b = ctp.tile([P, 4, TPC], BF, tag=f"ct{b}", name=f"ct{b}")
                nc.sync.dma_start(c_b[:], CT[b])
                ct_bs.append(c_b)
            wo_nxt = []
            for b in range(KB):
                w_b = wop.tile([P, 4, 512], BF, tag=f"wo{b}", name=f"wo{b}")
                nc.sync.dma_start(w_b[:], WO[b, 1])
                wo_nxt.append(w_b)
            # y_all accumulator (fp32); the residual XIN streams in as
            # per-(m,n) bf16 slices added at psum-eviction time, so its
            # bytes never compete with the lead-in wo/ct stream.
            y_all = yallp.tile([P, M, D], F32)
            nw_sb = None
            if not unit_nw:
                nw_sb = consts.tile([P, D], F32)
                nwap = NW[:]
                nw_bcast = bass.AP(
                    tensor=nwap.tensor, offset=nwap.offset, ap=[[0, P]] + list(nwap.ap)
                )
                nc.sync.dma_start(nw_sb[:], nw_bcast)
            eps_sb = consts.tile([P, 1], F32)
            nc.vector.memset(eps_sb[:], eps)

            ssm_t = ssmp.tile([P, M], F32)
            ss_m = [ssm_t[:, m : m + 1] for m in range(M)]

            def stats(m, n, y_slice):
                # incremental RMS stats: ss_m[m] += sum(y_slice^2)
                sq = sqp.tile([P, 512], F32, tag="sq", name="sq")
                ssp = statp.tile([P, 1], F32, tag="ssp", name="ssp")
                nc.scalar.activation(
                    sq[:],
                    y_slice,
                    mybir.ActivationFunctionType.Square,
                    accum_out=ssp[:],
                )
                if n == 0:
                    nc.vector.tensor_copy(ss_m[m], ssp[:])
                else:
                    nc.vector.tensor_add(ss_m[m], ss_m[m], ssp[:])

            def epilogue(m):
                y_m = y_all[:, m, :]
                rms = statp.tile([P, 1], F32, tag="rms", name="rms")
                nc.scalar.activation(
                    rms[:],
                    ss_m[m],
                    mybir.ActivationFunctionType.Sqrt,
                    bias=eps_sb[:],
                    scale=1.0 / D,
                )
                rstd = statp.tile([P, 1], F32, tag="rstd", name="rstd")
                nc.vector.reciprocal(rstd[:], rms[:])
                yn = ynp.tile([P, D], BF, tag="yn", name="yn")
                for h in range(1):
                    hs = slice(0, D)
                    if unit_nw:
                        nc.scalar.activation(
                            yn[:, hs],
                            y_all[:, m, hs],
                            mybir.ActivationFunctionType.Identity,
                            bias=0.0,
                            scale=rstd[:],
                        )
                    else:
                        nc.vector.scalar_tensor_tensor(
                            yn[:, hs],
                            y_all[:, m, hs],
                            rstd[:],
                            nw_sb[:, hs],
                            op0=mybir.AluOpType.mult,
                            op1=mybir.AluOpType.mult,
                        )
                    nc.sync.dma_start(Y[m * P : (m + 1) * P, hs], yn[:, hs])

            pss = [psp.tile([P, 512], F32, tag=f"ps{m}", name=f"ps{m}")
                   for m in range(M)]

            # PE warm-up during the DMA lead-in (see build_l1)
            warm = consts.tile([P, 64], BF)
            nc.vector.memset(warm[:], 1.0)
            for _ in range(48):
                nc.tensor.matmul(pss[0][0:64, 0:64], lhsT=warm[:, 0:64],
                                 rhs=warm[:], start=True, stop=True)

            for n in range(NC4):
                n0 = n * 512
                xin_ts = []
                for m in range(M):
                    xt_ = sqp.tile([P, 512], F32, tag=f"xin{m}", name=f"xin{m}")
                    nc.sync.dma_start(xt_[:], XINr[:, m, n0 : n0 + 512])
                    xin_ts.append(xt_)
                if n + 2 < NC4:
                    wo_n2 = []
                    for b in range(KB):
                        w_b = wop.tile([P, 4, 512], BF, tag=f"wo{b}", name=f"wo{b}")
                        nc.sync.dma_start(w_b[:], WO[b, n + 2])
                        wo_n2.append(w_b)
                if n == 0:
                    # k-outer for the first chunk: consumes each (wo, ct)
                    # bundle over 8 matmuls, pacing the k-loop to the DMA
                    # stream instead of stalling m=0 on the full 6MB.
                    for k in range(K):
                        for m in range(M):
                            nc.tensor.matmul(
                                pss[m][:],
                                lhsT=ct_bs[k // 4][:, k % 4, m * P : (m + 1) * P],
                                rhs=wo_cur[k // 4][:, k % 4, :],
                                start=(k == 0),
                                stop=(k == K - 1),
                            )
                    for m in range(M):
                        y_slice = y_all[:, m, n0 : n0 + 512]
                        nc.vector.tensor_add(y_slice, xin_ts[m][:], pss[m][:])
                        stats(m, n, y_slice)
                else:
                    # m-outer, k-inner: 16 consecutive matmuls accumulate
                    # into one PSUM bank before it's read (avoids psum-queue
                    # depth-cycling micro-idles).
                    for m in range(M):
                        for k in range(K):
                            nc.tensor.matmul(
                                pss[m][:],
                                lhsT=ct_bs[k // 4][:, k % 4, m * P : (m + 1) * P],
                                rhs=wo_cur[k // 4][:, k % 4, :],
                                start=(k == 0),
                                stop=(k == K - 1),
                            )
                        y_slice = y_all[:, m, n0 : n0 + 512]
                        nc.vector.tensor_add(y_slice, xin_ts[m][:], pss[m][:])
                        stats(m, n, y_slice)
                        if n + 1 == NC4:
                            # chain each m's epilogue behind its own k-loop
                            # so only m=M-1's trails the final matmul
                            epilogue(m)
                if n + 1 < NC4:
                    wo_cur = wo_nxt
                    if n + 2 < NC4:
                        wo_nxt = wo_n2
    nc.compile()
    return nc


def host_dispatch(xf, Wr, br):
    """Router + top-2 + softmax gates + expert grouping. Returns dispatch info."""
    T, D = xf.shape
    E = Wr.shape[1]
    logits = xf @ Wr + br
    i1 = np.argmax(logits, axis=1)
    l2 = logits.copy()
    l2[np.arange(T), i1] = -np.inf
    i2 = np.argmax(l2, axis=1)
    v1 = logits[np.arange(T), i1]
    v2 = logits[np.arange(T), i2]
    e2 = np.exp(v2 - v1)
    g1 = (1.0 / (1.0 + e2)).astype(np.float32)
    g2 = (e2 / (1.0 + e2)).astype(np.float32)

    # flat slots (t,s) grouped by expert, stable by (token, slot)
    ee = np.stack([i1, i2], 1).ravel()          # [2T]
    gg = np.stack([g1, g2], 1).ravel()
    tt = np.repeat(np.arange(T), 2)
    order = np.argsort(ee, kind="stable")
    counts = np.bincount(ee, minlength=E)
    starts = np.concatenate([[0], np.cumsum(counts)[:-1]])
    rank = np.empty(2 * T, np.int64)
    rank[order] = np.arange(2 * T)
    pos = rank - starts[ee]                      # position within expert's list
    return dict(
        e1=i1, e2=i2, counts=counts, order=order, starts=starts,
        pos=pos.reshape(T, 2), tok=tt, gate=gg, g1=g1, g2=g2,
    )


def bcap_for(counts):
    return int(np.ceil(max(int(counts.max()), 512) / 128) * 128)


def prep_l1_inputs(xf, d, We, be):
    """Per-expert L1 inputs: gathered+pretiled bf16 XT, bf16 W, fp32 be.

    XT0 holds the first 384 tokens (small lead-in chunk); XT holds the
    remainder re-tiled into 512-column slabs.
    """
    T, D = xf.shape
    E = We.shape[0]
    K = D // P
    F = D // P
    counts = d["counts"]
    Bcap = bcap_for(counts)
    Crest = len(_chunks(Bcap - 384, 512))
    Rpad = Crest * 512
    We_f = np.asarray(We, np.float32)
    be_f = np.asarray(be, np.float32)
    in1 = []
    for e in range(E):
        sel = d["order"][d["starts"][e] : d["starts"][e] + counts[e]]
        Xg = np.zeros((384 + Rpad, D), np.float32)
        Xg[: counts[e]] = xf[d["tok"][sel]]
        # [P, K, n]: contiguous per-partition DMA runs
        XT0 = np.ascontiguousarray(
            Xg[:384].reshape(1, 384, K, P).transpose(0, 3, 2, 1)[0]
        ).astype(BF16)
        XT_T = np.ascontiguousarray(
            Xg[384:].reshape(Crest, 512, K, P).transpose(0, 3, 2, 1)
        ).astype(BF16)
        W_T = np.ascontiguousarray(
            We_f[e].reshape(K, P, F, P).transpose(2, 1, 0, 3)
        ).astype(BF16)
        in1.append({"XT0": XT0, "XT": XT_T, "W": W_T, "BE": be_f[e]})
    return in1, Bcap


def prep_l2_inputs(xf, d, H, Wo, bo, norm_w):
    """Per-core L2 inputs. CT = g1*A + g2*B combined on host (fp32 math,
    one bf16 downcast); XIN = x + bo in fp32."""
    T, D = xf.shape
    TPC = T // NCORE
    KB = D // P // 4
    NC4 = D // 512
    # pretile Wo into contiguous (k-bundle, n-chunk) blocks
    Wo_b = np.ascontiguousarray(
        np.asarray(Wo, np.float32)
        .reshape(KB, 4, P, NC4, 512)
        .transpose(0, 3, 2, 1, 4)
    ).astype(BF16)
    bo_f = np.asarray(bo, np.float32)
    nw_f = np.asarray(norm_w, np.float32)
    e1, e2, pos = d["e1"], d["e2"], d["pos"]
    g1, g2 = d["g1"], d["g2"]
    in2 = []
    for c in range(NCORE):
        tl = np.arange(c * TPC, (c + 1) * TPC)
        CTf = np.empty((D, TPC), np.float32)
        BTf = np.empty((D, TPC), np.float32)
        for e in range(H.shape[0]):
            s1 = e1[tl] == e
            if s1.any():
                CTf[:, s1] = H[e][:, pos[tl[s1], 0]]
            s2 = e2[tl] == e
            if s2.any():
                BTf[:, s2] = H[e][:, pos[tl[s2], 1]]
        CTf = CTf * g1[tl][None, :] + BTf * g2[tl][None, :]
        CTt = np.ascontiguousarray(
            CTf.reshape(KB, 4, P, TPC).transpose(0, 2, 1, 3)
        ).astype(BF16)
        XIN = xf[tl] + bo_f[None, :]
        in2.append({"CT": CTt, "XIN": XIN, "WO": Wo_b, "NW": nw_f})
    return in2


# ----------------------------------------------------------------------------
# Harness entry point: full (unsharded) inputs -> full output.
# ----------------------------------------------------------------------------
_L1_CACHE = {}
_L2_CACHE = {}


def kernel(x, Wr, br, We, be, Wo, bo, norm_w):
    B, S, D = x.shape
    E = We.shape[0]
    T = B * S
    TPC = T // NCORE
    xf = np.ascontiguousarray(np.asarray(x, np.float32).reshape(T, D))
    d = host_dispatch(xf, np.asarray(Wr, np.float32), np.asarray(br, np.float32))

    in1, Bcap = prep_l1_inputs(xf, d, We, be)
    if (D, Bcap) not in _L1_CACHE:
        _L1_CACHE[(D, Bcap)] = build_l1(D, Bcap)
    r1 = run_bass_kernel_spmd(_L1_CACHE[(D, Bcap)], in1, list(range(NCORE)))
    H = np.stack([r1.results[e]["H"] for e in range(E)])

    in2 = prep_l2_inputs(xf, d, H, Wo, bo, norm_w)
    unit_nw = bool(np.all(np.asarray(norm_w, np.float32) == 1.0))
    if (D, TPC, unit_nw) not in _L2_CACHE:
        _L2_CACHE[(D, TPC, unit_nw)] = build_l2(D, TPC, unit_nw)
    r2 = run_bass_kernel_spmd(_L2_CACHE[(D, TPC, unit_nw)], in2, list(range(NCORE)))
    Y = np.concatenate([r2.results[c]["Y"] for c in range(NCORE)], axis=0)
    return Y.reshape(B, S, D).astype(np.asarray(x).dtype)


# revision 50
# speedup vs baseline: 1.0145x; 1.0145x over previous
# MoE EnhancedGatedFusion kernel for 8x TRN2 NeuronCores (expert-parallel).
#
# Decomposition:
#   host : router logits -> top2 -> softmax gates -> dispatch by expert
#   L1   : per-core (expert e): H_T[d_out, n] = silu(We[e].T-contract @ XT + be[e])
#          (ungated; bf16 matmul operands, fp32 PSUM + fp32 H output)
#   host : combine CT = g1*A + g2*B during the token un-shuffle (the
#          "all-to-all return" glue), downcast once to bf16.
#   L2   : per-core (1024 tokens): out = CT.T @ Wo; y = XIN + out (XIN =
#          x_shard + bo folded on host, fp32); RMS-norm * norm_w.
#
# Matmul operands are bf16: same 1 cycle/row PE rate as fp32r but half the
# HBM/SBUF traffic, and FWL (fast weight load) kicks in for non-fp32 dtypes
# so LDWEIGHTS hides under the 64-deep PE reorder window.
import sys
import types

sys.path.insert(0, "/opt/trn_rl_repo")

import numpy as np
import ml_dtypes

BF16 = np.dtype(ml_dtypes.bfloat16)


def _install_ntff_hook():
    # antenv.axon_hooks is missing in this image; shim it so
    # run_bass_kernel_spmd(trace=True) can drive NTFF profiling.
    if "antenv.axon_hooks" in sys.modules:
        return
    try:
        from trn_agent_boot.trn_boot import _ntff_profile_via_ctypes

        hook = _ntff_profile_via_ctypes("/opt/axon/libaxon_pjrt.so")
    except Exception:
        hook = None
    mod = types.ModuleType("antenv.axon_hooks")
    mod.get_axon_ntff_profile_hook = lambda: hook
    mod.set_axon_ntff_profile_hook = lambda h: None
    sys.modules["antenv.axon_hooks"] = mod


_install_ntff_hook()

import concourse.bacc as bacc
import concourse.bass as bass
import concourse.tile as tile
from concourse import mybir
from concourse.bass_utils import run_bass_kernel_spmd

F32 = mybir.dt.float32
BF = mybir.dt.bfloat16
P = 128
NCORE = 8


def _chunks(total, size):
    out = []
    o = 0
    while o < total:
        out.append((o, min(size, total - o)))
        o += size
    return out


def build_l1(D, Bcap):
    """Per-core expert FFN: H[d_out, n] = silu(sum_k W[k,d_out]*XT[k,n] + be[d_out]).

    XT_T is host-pretiled bf16 [C, P, K, 512] (zero-padded ragged tail) and
    W_T is bf16 [F, P, K, P]; W is fully SBUF-resident (8.4MB), XT streams
    through once.  H output is fp32.

    The first 512-slab is processed as two 256-col sub-chunks so the
    lead-in only gates on 1MB of XT + the first W tile; remaining W tiles
    stream behind while the f-loop burns through the small chunk.
    """
    K = D // P          # k-tiles
    F = D // P          # feat (d_out) tiles
    chunks = [(0, 384)] + [(384 + o, n) for o, n in _chunks(Bcap - 384, 512)]
    Crest = len(chunks) - 1
    nc = bacc.Bacc("TRN2", target_bir_lowering=False, debug=False)
    XT0 = nc.dram_tensor("XT0", [P, K, 384], BF, kind="ExternalInput")
    XT = nc.dram_tensor("XT", [Crest, P, K, 512], BF, kind="ExternalInput")
    W = nc.dram_tensor("W", [F, P, K, P], BF, kind="ExternalInput")
    BE = nc.dram_tensor("BE", [D], F32, kind="ExternalInput")
    H = nc.dram_tensor("H", [D, Bcap], F32, kind="ExternalOutput")

    Hr = H[:, :].rearrange("(fo p) n -> p fo n", p=P)

    with tile.TileContext(nc) as tc:
        with (
            tc.tile_pool(name="consts", bufs=1) as consts,
            tc.tile_pool(name="xt", bufs=3) as xtp,
            tc.tile_pool(name="wf", bufs=1) as wfp,
            tc.tile_pool(name="hout", bufs=4) as hp,
            tc.tile_pool(name="ps", bufs=4, space="PSUM") as psp,
        ):
            # DMA issue order tuned for the lead-in: w0, be, first 256 cols
            # of XT, then remaining W tiles (second sub-chunk's XT slotted
            # midway so it lands before the f-loop reaches it).
            # PE warm-up: ~48 tiny matmuls during the DMA lead-in keep the
            # HAM activity window busy so real matmuls start at 2.4GHz.
            warm = consts.tile([P, 64], BF)
            nc.vector.memset(warm[:], 1.0)
            wps = psp.tile([P, 512], F32, tag="ps", name="ps")
            for _ in range(48):
                nc.tensor.matmul(wps[0:64, 0:64], lhsT=warm[:, 0:64],
                                 rhs=warm[:], start=True, stop=True)

            w_tiles = [None] * F
            w_tiles[0] = wfp.tile([P, K, P], BF, tag="wf0", name="wf0")
            nc.sync.dma_start(w_tiles[0][:, 0 : K // 2, :], W[0, :, 0 : K // 2, :])
            xt_tiles = {}
            xt_tiles[0] = xtp.tile([P, K, 512], BF, tag="xt", name="xt")
            nc.sync.dma_start(xt_tiles[0][:, 0 : K // 2, 0:384],
                              XT0[:, 0 : K // 2, :])
            nc.sync.dma_start(w_tiles[0][:, K // 2 : K, :], W[0, :, K // 2 : K, :])
            nc.sync.dma_start(xt_tiles[0][:, K // 2 : K, 0:384],
                              XT0[:, K // 2 : K, :])
            be_sb = consts.tile([P, F], F32)
            nc.sync.dma_start(be_sb[:], BE[:].rearrange("(f p) -> p f", p=P))
            for f in range(1, F):
                w_f = wfp.tile([P, K, P], BF, tag=f"wf{f}", name=f"wf{f}")
                nc.sync.dma_start(w_f[:], W[f])
                w_tiles[f] = w_f

            for ci, (c0, cn) in enumerate(chunks):
                if ci in xt_tiles:
                    xt_c = xt_tiles[ci]
                else:
                    xt_c = xtp.tile([P, K, 512], BF, tag="xt", name="xt")
                    nc.sync.dma_start(xt_c[:, :, :cn],
                                      XT[ci - 1, :, :, :cn])
                for f in range(F):
                    ps = psp.tile([P, 512], F32, tag="ps", name="ps")
                    for k in range(K):
                        nc.tensor.matmul(
                            ps[:, :cn],
                            lhsT=w_tiles[f][:, k, :],
                            rhs=xt_c[:, k, :cn],
                            start=(k == 0),
                            stop=(k == K - 1),
                        )
                    h_t = hp.tile([P, 512], F32, tag="h", name="h")
                    nc.scalar.activation(
                        h_t[:, :cn],
                        ps[:, :cn],
                        mybir.ActivationFunctionType.Silu,
                        bias=be_sb[:, f : f + 1],
                        scale=1.0,
                    )
                    nc.sync.dma_start(Hr[:, f, c0 : c0 + cn], h_t[:, :cn])
    nc.compile()
    return nc


def build_l2(D, TPC, unit_nw, eps=1e-6):
    """Per-core combine + output proj + residual + RMS norm.

    Y[t, j] = nw[j] * (XIN[t,j] + sum_k CT[k,t]*Wo[k,j]) / rms(t)
    CT = g1*A + g2*B (host-combined, bf16); XIN = x_shard + bo (fp32).
    Y output is bf16 (host upcasts).

    The last n-chunk runs m-outer so each m's epilogue (RMS + scale +
    store) chains behind its own k-loop and overlaps the next m's
    matmuls; only m=M-1's epilogue trails the final matmul.

    unit_nw=True specializes norm_w == 1 (scale-by-rstd runs as a scalar
    engine activation; multiplying by 1 is exact) so the vector engine
    stays under the per-m tensor budget during the epilogue phase.
    """
    K = D // P
    M = TPC // P
    NC4 = D // 512
    KB = K // 4          # k-tiles bundled per DMA
    nc = bacc.Bacc("TRN2", target_bir_lowering=False, debug=False)
    # CT/WO are host-pretiled so every bundle DMA reads long contiguous
    # per-partition runs (8KB / 4KB) — strided reads from the natural
    # [D, x] layout only sustain ~40% of DMA bandwidth and gate the lead-in.
    CT = nc.dram_tensor("CT", [KB, P, 4, TPC], BF, kind="ExternalInput")
    XIN = nc.dram_tensor("XIN", [TPC, D], F32, kind="ExternalInput")
    WO = nc.dram_tensor("WO", [KB, NC4, P, 4, 512], BF, kind="ExternalInput")
    NW = nc.dram_tensor("NW", [D], F32, kind="ExternalInput")
    Y = nc.dram_tensor("Y", [TPC, D], BF, kind="ExternalOutput")

    XINr = XIN[:, :].rearrange("(m p) d -> p m d", p=P)

    with tile.TileContext(nc) as tc:
        with (
            tc.tile_pool(name="consts", bufs=1) as consts,
            tc.tile_pool(name="ct", bufs=1) as ctp,
            tc.tile_pool(name="wo", bufs=3) as wop,
            tc.tile_pool(name="yall", bufs=1) as yallp,
            tc.tile_pool(name="sq", bufs=3) as sqp,
            tc.tile_pool(name="yn", bufs=2) as ynp,
            tc.tile_pool(name="ssm", bufs=1) as ssmp,
            tc.tile_pool(name="stat", bufs=4) as statp,
            tc.tile_pool(name="ps", bufs=1, space="PSUM") as psp,
        ):
            # Bundled DMAs (4 k-tiles each), interleaved (wo, ct) pairs so
            # the k-loop's operands arrive in consumption order; XIN and nw
            # are queued behind everything n=0/n=1 needs.
            ct_bs = []
            wo_cur = []
            for b in range(KB):
                w_b = wop.tile([P, 4, 512], BF, tag=f"wo{b}", name=f"wo{b}")
                nc.sync.dma_start(w_b[:], WO[b, 0])
                wo_cur.append(w_b)
                c_b = ctp.tile([P, 4, TPC], BF, tag=f"ct{b}", name=f"ct{b}")
                nc.sync.dma_start(c_b[:], CT[b])
                ct_bs.append(c_b)
            wo_nxt = []
            for b in range(KB):
                w_b = wop.tile([P, 4, 512], BF, tag=f"wo{b}", name=f"wo{b}")
                nc.sync.dma_start(w_b[:], WO[b, 1])
                wo_nxt.append(w_b)
            # y_all accumulator (fp32); the residual XIN streams in as
            # per-(m,n) bf16 slices added at psum-eviction time, so its
            # bytes never compete with the lead-in wo/ct stream.
            y_all = yallp.tile([P, M, D], F32)
            nw_sb = None
            if not unit_nw:
                nw_sb = consts.tile([P, D], F32)
                nwap = NW[:]
                nw_bcast = bass.AP(
                    tensor=nwap.tensor, offset=nwap.offset, ap=[[0, P]] + list(nwap.ap)
                )
                nc.sync.dma_start(nw_sb[:], nw_bcast)
            eps_sb = consts.tile([P, 1], F32)
            nc.vector.memset(eps_sb[:], eps)

            ssm_t = ssmp.tile([P, M], F32)
            ss_m = [ssm_t[:, m : m + 1] for m in range(M)]

            def stats(m, n, y_slice):
                # incremental RMS stats: ss_m[m] += sum(y_slice^2)
                sq = sqp.tile([P, 512], F32, tag="sq", name="sq")
                ssp = statp.tile([P, 1], F32, tag="ssp", name="ssp")
                nc.scalar.activation(
                    sq[:],
                    y_slice,
                    mybir.ActivationFunctionType.Square,
                    accum_out=ssp[:],
                )
                if n == 0:
                    nc.vector.tensor_copy(ss_m[m], ssp[:])
                else:
                    nc.vector.tensor_add(ss_m[m], ss_m[m], ssp[:])

            def epilogue(m):
                y_m = y_all[:, m, :]
                rms = statp.tile([P, 1], F32, tag="rms", name="rms")
                nc.scalar.activation(
                    rms[:],
                    ss_m[m],
                    mybir.ActivationFunctionType.Sqrt,
                    bias=eps_sb[:],
                    scale=1.0 / D,
                )
                rstd = statp.tile([P, 1], F32, tag="rstd", name="rstd")
                nc.vector.reciprocal(rstd[:], rms[:])
                yn = ynp.tile([P, D], BF, tag="yn", name="yn")
                for h in range(1):
                    hs = slice(0, D)
                    if unit_nw:
                        # alternate engines by m so neither scalar nor
                        # vector backlogs behind the 3.4us/m tensor pace
                        if m % 2 == 0:
                            nc.scalar.activation(
                                yn[:, hs],
                                y_all[:, m, hs],
                                mybir.ActivationFunctionType.Identity,
                                bias=0.0,
                                scale=rstd[:],
                            )
                        else:
                            nc.vector.tensor_scalar_mul(
                                yn[:, hs], y_all[:, m, hs], rstd[:]
                            )
                    else:
                        nc.vector.scalar_tensor_tensor(
                            yn[:, hs],
                            y_all[:, m, hs],
                            rstd[:],
                            nw_sb[:, hs],
                            op0=mybir.AluOpType.mult,
                            op1=mybir.AluOpType.mult,
                        )
                    nc.sync.dma_start(Y[m * P : (m + 1) * P, hs], yn[:, hs])

            pss = [psp.tile([P, 512], F32, tag=f"ps{m}", name=f"ps{m}")
                   for m in range(M)]

            # PE warm-up during the DMA lead-in (see build_l1)
            warm = consts.tile([P, 64], BF)
            nc.vector.memset(warm[:], 1.0)
            for _ in range(48):
                nc.tensor.matmul(pss[0][0:64, 0:64], lhsT=warm[:, 0:64],
                                 rhs=warm[:], start=True, stop=True)

            for n in range(NC4):
                n0 = n * 512
                xin_ts = []
                for m in range(M):
                    xt_ = sqp.tile([P, 512], F32, tag=f"xin{m}", name=f"xin{m}")
                    nc.sync.dma_start(xt_[:], XINr[:, m, n0 : n0 + 512])
                    xin_ts.append(xt_)
                if n + 2 < NC4:
                    wo_n2 = []
                    for b in range(KB):
                        w_b = wop.tile([P, 4, 512], BF, tag=f"wo{b}", name=f"wo{b}")
                        nc.sync.dma_start(w_b[:], WO[b, n + 2])
                        wo_n2.append(w_b)
                if n == 0:
                    # k-outer for the first chunk: consumes each (wo, ct)
                    # bundle over 8 matmuls, pacing the k-loop to the DMA
                    # stream instead of stalling m=0 on the full 6MB.
                    for k in range(K):
                        for m in range(M):
                            nc.tensor.matmul(
                                pss[m][:],
                                lhsT=ct_bs[k // 4][:, k % 4, m * P : (m + 1) * P],
                                rhs=wo_cur[k // 4][:, k % 4, :],
                                start=(k == 0),
                                stop=(k == K - 1),
                            )
                    for m in range(M):
                        y_slice = y_all[:, m, n0 : n0 + 512]
                        nc.vector.tensor_add(y_slice, xin_ts[m][:], pss[m][:])
                        stats(m, n, y_slice)
                else:
                    # m-outer, k-inner: 16 consecutive matmuls accumulate
                    # into one PSUM bank before it's read (avoids psum-queue
                    # depth-cycling micro-idles).
                    for m in range(M):
                        for k in range(K):
                            nc.tensor.matmul(
                                pss[m][:],
                                lhsT=ct_bs[k // 4][:, k % 4, m * P : (m + 1) * P],
                                rhs=wo_cur[k // 4][:, k % 4, :],
                                start=(k == 0),
                                stop=(k == K - 1),
                            )
                        y_slice = y_all[:, m, n0 : n0 + 512]
                        nc.vector.tensor_add(y_slice, xin_ts[m][:], pss[m][:])
                        stats(m, n, y_slice)
                        if n + 1 == NC4:
                            # chain each m's epilogue behind its own k-loop
                            # so only m=M-1's trails the final matmul
                            epilogue(m)
                if n + 1 < NC4:
                    wo_cur = wo_nxt
                    if n + 2 < NC4:
                        wo_nxt = wo_n2
    nc.compile()
    return nc


def host_dispatch(xf, Wr, br):
    """Router + top-2 + softmax gates + expert grouping. Returns dispatch info."""
    T, D = xf.shape
    E = Wr.shape[1]
    logits = xf @ Wr + br
    i1 = np.argmax(logits, axis=1)
    l2 = logits.copy()
    l2[np.arange(T), i1] = -np.inf
    i2 = np.argmax(l2, axis=1)
    v1 = logits[np.arange(T), i1]
    v2 = logits[np.arange(T), i2]
    e2 = np.exp(v2 - v1)
    g1 = (1.0 / (1.0 + e2)).astype(np.float32)
    g2 = (e2 / (1.0 + e2)).astype(np.float32)

    # flat slots (t,s) grouped by expert, stable by (token, slot)
    ee = np.stack([i1, i2], 1).ravel()          # [2T]
    gg = np.stack([g1, g2], 1).ravel()
    tt = np.repeat(np.arange(T), 2)
    order = np.argsort(ee, kind="stable")
    counts = np.bincount(ee, minlength=E)
    starts = np.concatenate([[0], np.cumsum(counts)[:-1]])
    rank = np.empty(2 * T, np.int64)
    rank[order] = np.arange(2 * T)
    pos = rank - starts[ee]                      # position within expert's list
    return dict(
        e1=i1, e2=i2, counts=counts, order=order, starts=starts,
        pos=pos.reshape(T, 2), tok=tt, gate=gg, g1=g1, g2=g2,
    )


def bcap_for(counts):
    return int(np.ceil(max(int(counts.max()), 512) / 128) * 128)


def prep_l1_inputs(xf, d, We, be):
    """Per-expert L1 inputs: gathered+pretiled bf16 XT, bf16 W, fp32 be.

    XT0 holds the first 384 tokens (small lead-in chunk); XT holds the
    remainder re-tiled into 512-column slabs.
    """
    T, D = xf.shape
    E = We.shape[0]
    K = D // P
    F = D // P
    counts = d["counts"]
    Bcap = bcap_for(counts)
    Crest = len(_chunks(Bcap - 384, 512))
    Rpad = Crest * 512
    We_f = np.asarray(We, np.float32)
    be_f = np.asarray(be, np.float32)
    in1 = []
    for e in range(E):
        sel = d["order"][d["starts"][e] : d["starts"][e] + counts[e]]
        Xg = np.zeros((384 + Rpad, D), np.float32)
        Xg[: counts[e]] = xf[d["tok"][sel]]
        # [P, K, n]: contiguous per-partition DMA runs
        XT0 = np.ascontiguousarray(
            Xg[:384].reshape(1, 384, K, P).transpose(0, 3, 2, 1)[0]
        ).astype(BF16)
        XT_T = np.ascontiguousarray(
            Xg[384:].reshape(Crest, 512, K, P).transpose(0, 3, 2, 1)
        ).astype(BF16)
        W_T = np.ascontiguousarray(
            We_f[e].reshape(K, P, F, P).transpose(2, 1, 0, 3)
        ).astype(BF16)
        in1.append({"XT0": XT0, "XT": XT_T, "W": W_T, "BE": be_f[e]})
    return in1, Bcap


def prep_l2_inputs(xf, d, H, Wo, bo, norm_w):
    """Per-core L2 inputs. CT = g1*A + g2*B combined on host (fp32 math,
    one bf16 downcast); XIN = x + bo in fp32."""
    T, D = xf.shape
    TPC = T // NCORE
    KB = D // P // 4
    NC4 = D // 512
    # pretile Wo into contiguous (k-bundle, n-chunk) blocks
    Wo_b = np.ascontiguousarray(
        np.asarray(Wo, np.float32)
        .reshape(KB, 4, P, NC4, 512)
        .transpose(0, 3, 2, 1, 4)
    ).astype(BF16)
    bo_f = np.asarray(bo, np.float32)
    nw_f = np.asarray(norm_w, np.float32)
    e1, e2, pos = d["e1"], d["e2"], d["pos"]
    g1, g2 = d["g1"], d["g2"]
    in2 = []
    for c in range(NCORE):
        tl = np.arange(c * TPC, (c + 1) * TPC)
        CTf = np.empty((D, TPC), np.float32)
        BTf = np.empty((D, TPC), np.float32)
        for e in range(H.shape[0]):
            s1 = e1[tl] == e
            if s1.any():
                CTf[:, s1] = H[e][:, pos[tl[s1], 0]]
            s2 = e2[tl] == e
            if s2.any():
                BTf[:, s2] = H[e][:, pos[tl[s2], 1]]
        CTf = CTf * g1[tl][None, :] + BTf * g2[tl][None, :]
        CTt = np.ascontiguousarray(
            CTf.reshape(KB, 4, P, TPC).transpose(0, 2, 1, 3)
        ).astype(BF16)
        XIN = xf[tl] + bo_f[None, :]
        in2.append({"CT": CTt, "XIN": XIN, "WO": Wo_b, "NW": nw_f})
    return in2


# ----------------------------------------------------------------------------
# Harness entry point: full (unsharded) inputs -> full output.
# ----------------------------------------------------------------------------
_L1_CACHE = {}
_L2_CACHE = {}


def kernel(x, Wr, br, We, be, Wo, bo, norm_w):
    B, S, D = x.shape
    E = We.shape[0]
    T = B * S
    TPC = T // NCORE
    xf = np.ascontiguousarray(np.asarray(x, np.float32).reshape(T, D))
    d = host_dispatch(xf, np.asarray(Wr, np.float32), np.asarray(br, np.float32))

    in1, Bcap = prep_l1_inputs(xf, d, We, be)
    if (D, Bcap) not in _L1_CACHE:
        _L1_CACHE[(D, Bcap)] = build_l1(D, Bcap)
    r1 = run_bass_kernel_spmd(_L1_CACHE[(D, Bcap)], in1, list(range(NCORE)))
    H = np.stack([r1.results[e]["H"] for e in range(E)])

    in2 = prep_l2_inputs(xf, d, H, Wo, bo, norm_w)
    unit_nw = bool(np.all(np.asarray(norm_w, np.float32) == 1.0))
    if (D, TPC, unit_nw) not in _L2_CACHE:
        _L2_CACHE[(D, TPC, unit_nw)] = build_l2(D, TPC, unit_nw)
    r2 = run_bass_kernel_spmd(_L2_CACHE[(D, TPC, unit_nw)], in2, list(range(NCORE)))
    Y = np.concatenate([r2.results[c]["Y"] for c in range(NCORE)], axis=0)
    return Y.reshape(B, S, D).astype(np.asarray(x).dtype)


# revision 56
# speedup vs baseline: 1.0220x; 1.0074x over previous
# MoE EnhancedGatedFusion kernel for 8x TRN2 NeuronCores (expert-parallel).
#
# Decomposition:
#   host : router logits -> top2 -> softmax gates -> dispatch by expert
#   L1   : per-core (expert e): H_T[d_out, n] = silu(We[e].T-contract @ XT + be[e])
#          (ungated; bf16 matmul operands, fp32 PSUM + fp32 H output)
#   host : combine CT = g1*A + g2*B during the token un-shuffle (the
#          "all-to-all return" glue), downcast once to bf16.
#   L2   : per-core (1024 tokens): out = CT.T @ Wo; y = XIN + out (XIN =
#          x_shard + bo folded on host, fp32); RMS-norm * norm_w.
#
# Matmul operands are bf16: same 1 cycle/row PE rate as fp32r but half the
# HBM/SBUF traffic, and FWL (fast weight load) kicks in for non-fp32 dtypes
# so LDWEIGHTS hides under the 64-deep PE reorder window.
import sys
import types

sys.path.insert(0, "/opt/trn_rl_repo")

import numpy as np
import ml_dtypes

BF16 = np.dtype(ml_dtypes.bfloat16)


def _install_ntff_hook():
    # antenv.axon_hooks is missing in this image; shim it so
    # run_bass_kernel_spmd(trace=True) can drive NTFF profiling.
    if "antenv.axon_hooks" in sys.modules:
        return
    try:
        from trn_agent_boot.trn_boot import _ntff_profile_via_ctypes

        hook = _ntff_profile_via_ctypes("/opt/axon/libaxon_pjrt.so")
    except Exception:
        hook = None
    mod = types.ModuleType("antenv.axon_hooks")
    mod.get_axon_ntff_profile_hook = lambda: hook
    mod.set_axon_ntff_profile_hook = lambda h: None
    sys.modules["antenv.axon_hooks"] = mod


_install_ntff_hook()

import concourse.bacc as bacc
import concourse.bass as bass
import concourse.tile as tile
from concourse import mybir
from concourse.bass_utils import run_bass_kernel_spmd

F32 = mybir.dt.float32
BF = mybir.dt.bfloat16
P = 128
NCORE = 8


def _chunks(total, size):
    out = []
    o = 0
    while o < total:
        out.append((o, min(size, total - o)))
        o += size
    return out


def build_l1(D, Bcap):
    """Per-core expert FFN: H[d_out, n] = silu(sum_k W[k,d_out]*XT[k,n] + be[d_out]).

    XT_T is host-pretiled bf16 [C, P, K, 512] (zero-padded ragged tail) and
    W_T is bf16 [F, P, K, P]; W is fully SBUF-resident (8.4MB), XT streams
    through once.  H output is fp32.

    The first 512-slab is processed as two 256-col sub-chunks so the
    lead-in only gates on 1MB of XT + the first W tile; remaining W tiles
    stream behind while the f-loop burns through the small chunk.
    """
    K = D // P          # k-tiles
    F = D // P          # feat (d_out) tiles
    chunks = [(0, 384)] + [(384 + o, n) for o, n in _chunks(Bcap - 384, 512)]
    Crest = len(chunks) - 1
    nc = bacc.Bacc("TRN2", target_bir_lowering=False, debug=False)
    XT0 = nc.dram_tensor("XT0", [P, K, 384], BF, kind="ExternalInput")
    XT = nc.dram_tensor("XT", [Crest, P, K, 512], BF, kind="ExternalInput")
    W = nc.dram_tensor("W", [F, P, K, P], BF, kind="ExternalInput")
    BE = nc.dram_tensor("BE", [D], F32, kind="ExternalInput")
    # H is bf16: DMA runs packet-overhead-limited at ~155GB/s effective,
    # so halving the 17.8MB H write materially unloads the engines
    H = nc.dram_tensor("H", [D, Bcap], BF, kind="ExternalOutput")

    Hr = H[:, :].rearrange("(fo p) n -> p fo n", p=P)

    with tile.TileContext(nc) as tc:
        with (
            tc.tile_pool(name="consts", bufs=1) as consts,
            tc.tile_pool(name="xt", bufs=3) as xtp,
            tc.tile_pool(name="wf", bufs=1) as wfp,
            tc.tile_pool(name="hout", bufs=4) as hp,
            tc.tile_pool(name="ps", bufs=4, space="PSUM") as psp,
        ):
            # DMA issue order tuned for the lead-in: w0, be, first 256 cols
            # of XT, then remaining W tiles (second sub-chunk's XT slotted
            # midway so it lands before the f-loop reaches it).
            # PE warm-up: ~48 tiny matmuls during the DMA lead-in keep the
            # HAM activity window busy so real matmuls start at 2.4GHz.
            warm = consts.tile([P, 64], BF)
            nc.vector.memset(warm[:], 1.0)
            wps = psp.tile([P, 512], F32, tag="ps", name="ps")
            for _ in range(48):
                nc.tensor.matmul(wps[0:64, 0:64], lhsT=warm[:, 0:64],
                                 rhs=warm[:], start=True, stop=True)

            w_tiles = [None] * F
            w_tiles[0] = wfp.tile([P, K, P], BF, tag="wf0", name="wf0")
            nc.sync.dma_start(w_tiles[0][:, 0 : K // 2, :], W[0, :, 0 : K // 2, :])
            xt_tiles = {}
            xt_tiles[0] = xtp.tile([P, K, 512], BF, tag="xt", name="xt")
            nc.sync.dma_start(xt_tiles[0][:, 0 : K // 2, 0:384],
                              XT0[:, 0 : K // 2, :])
            nc.sync.dma_start(w_tiles[0][:, K // 2 : K, :], W[0, :, K // 2 : K, :])
            nc.sync.dma_start(xt_tiles[0][:, K // 2 : K, 0:384],
                              XT0[:, K // 2 : K, :])
            be_sb = consts.tile([P, F], F32)
            nc.sync.dma_start(be_sb[:], BE[:].rearrange("(f p) -> p f", p=P))
            for f in range(1, F):
                w_f = wfp.tile([P, K, P], BF, tag=f"wf{f}", name=f"wf{f}")
                nc.sync.dma_start(w_f[:], W[f])
                w_tiles[f] = w_f

            for ci, (c0, cn) in enumerate(chunks):
                if ci in xt_tiles:
                    xt_c = xt_tiles[ci]
                else:
                    xt_c = xtp.tile([P, K, 512], BF, tag="xt", name="xt")
                    nc.sync.dma_start(xt_c[:, :, :cn],
                                      XT[ci - 1, :, :, :cn])
                for f in range(F):
                    ps = psp.tile([P, 512], F32, tag="ps", name="ps")
                    for k in range(K):
                        nc.tensor.matmul(
                            ps[:, :cn],
                            lhsT=w_tiles[f][:, k, :],
                            rhs=xt_c[:, k, :cn],
                            start=(k == 0),
                            stop=(k == K - 1),
                        )
                    h_t = hp.tile([P, 512], BF, tag="h", name="h")
                    nc.scalar.activation(
                        h_t[:, :cn],
                        ps[:, :cn],
                        mybir.ActivationFunctionType.Silu,
                        bias=be_sb[:, f : f + 1],
                        scale=1.0,
                    )
                    nc.sync.dma_start(Hr[:, f, c0 : c0 + cn], h_t[:, :cn])
    nc.compile()
    return nc


def build_l2(D, TPC, unit_nw, eps=1e-6):
    """Per-core combine + output proj + residual + RMS norm.

    Y[t, j] = nw[j] * (XIN[t,j] + sum_k CT[k,t]*Wo[k,j]) / rms(t)
    CT = g1*A + g2*B (host-combined, bf16); XIN = x_shard + bo (fp32).
    Y output is bf16 (host upcasts).

    The last n-chunk runs m-outer so each m's epilogue (RMS + scale +
    store) chains behind its own k-loop and overlaps the next m's
    matmuls; only m=M-1's epilogue trails the final matmul.

    unit_nw=True specializes norm_w == 1 (scale-by-rstd runs as a scalar
    engine activation; multiplying by 1 is exact) so the vector engine
    stays under the per-m tensor budget during the epilogue phase.
    """
    K = D // P
    M = TPC // P
    NC4 = D // 512
    KB = K // 4          # k-tiles bundled per DMA
    nc = bacc.Bacc("TRN2", target_bir_lowering=False, debug=False)
    # CT/WO are host-pretiled so every bundle DMA reads long contiguous
    # per-partition runs (8KB / 4KB) — strided reads from the natural
    # [D, x] layout only sustain ~40% of DMA bandwidth and gate the lead-in.
    CT = nc.dram_tensor("CT", [KB, P, 4, TPC], BF, kind="ExternalInput")
    XIN = nc.dram_tensor("XIN", [TPC, D], BF, kind="ExternalInput")
    WO = nc.dram_tensor("WO", [KB, NC4, P, 4, 512], BF, kind="ExternalInput")
    NW = nc.dram_tensor("NW", [D], F32, kind="ExternalInput")
    Y = nc.dram_tensor("Y", [TPC, D], BF, kind="ExternalOutput")

    XINr = XIN[:, :].rearrange("(m p) d -> p m d", p=P)

    with tile.TileContext(nc) as tc:
        with (
            tc.tile_pool(name="consts", bufs=1) as consts,
            tc.tile_pool(name="ct", bufs=1) as ctp,
            tc.tile_pool(name="wo", bufs=3) as wop,
            tc.tile_pool(name="yall", bufs=1) as yallp,
            tc.tile_pool(name="sq", bufs=3) as sqp,
            tc.tile_pool(name="yn", bufs=2) as ynp,
            tc.tile_pool(name="ssm", bufs=1) as ssmp,
            tc.tile_pool(name="stat", bufs=4) as statp,
            tc.tile_pool(name="ps", bufs=1, space="PSUM") as psp,
        ):
            # Bundled DMAs (4 k-tiles each), interleaved (wo, ct) pairs so
            # the k-loop's operands arrive in consumption order; XIN and nw
            # are queued behind everything n=0/n=1 needs.
            ct_bs = []
            wo_cur = []
            for b in range(KB):
                w_b = wop.tile([P, 4, 512], BF, tag=f"wo{b}", name=f"wo{b}")
                nc.sync.dma_start(w_b[:], WO[b, 0])
                wo_cur.append(w_b)
                c_b = ctp.tile([P, 4, TPC], BF, tag=f"ct{b}", name=f"ct{b}")
                nc.sync.dma_start(c_b[:], CT[b])
                ct_bs.append(c_b)
            wo_nxt = []
            for b in range(KB):
                w_b = wop.tile([P, 4, 512], BF, tag=f"wo{b}", name=f"wo{b}")
                nc.sync.dma_start(w_b[:], WO[b, 1])
                wo_nxt.append(w_b)
            # y_all accumulator (fp32); the residual XIN streams in as
            # per-(m,n) bf16 slices added at psum-eviction time, so its
            # bytes never compete with the lead-in wo/ct stream.
            y_all = yallp.tile([P, M, D], F32)
            nw_sb = None
            if not unit_nw:
                nw_sb = consts.tile([P, D], F32)
                nwap = NW[:]
                nw_bcast = bass.AP(
                    tensor=nwap.tensor, offset=nwap.offset, ap=[[0, P]] + list(nwap.ap)
                )
                nc.sync.dma_start(nw_sb[:], nw_bcast)
            eps_sb = consts.tile([P, 1], F32)
            nc.vector.memset(eps_sb[:], eps)

            ssm_t = ssmp.tile([P, M], F32)
            ss_m = [ssm_t[:, m : m + 1] for m in range(M)]

            def stats(m, n, y_slice):
                # incremental RMS stats: ss_m[m] += sum(y_slice^2)
                sq = sqp.tile([P, 512], F32, tag="sq", name="sq")
                ssp = statp.tile([P, 1], F32, tag="ssp", name="ssp")
                nc.scalar.activation(
                    sq[:],
                    y_slice,
                    mybir.ActivationFunctionType.Square,
                    accum_out=ssp[:],
                )
                if n == 0:
                    nc.vector.tensor_copy(ss_m[m], ssp[:])
                else:
                    nc.vector.tensor_add(ss_m[m], ss_m[m], ssp[:])

            def epilogue(m):
                y_m = y_all[:, m, :]
                rms = statp.tile([P, 1], F32, tag="rms", name="rms")
                nc.scalar.activation(
                    rms[:],
                    ss_m[m],
                    mybir.ActivationFunctionType.Sqrt,
                    bias=eps_sb[:],
                    scale=1.0 / D,
                )
                rstd = statp.tile([P, 1], F32, tag="rstd", name="rstd")
                nc.vector.reciprocal(rstd[:], rms[:])
                yn = ynp.tile([P, D], BF, tag="yn", name="yn")
                for h in range(1):
                    hs = slice(0, D)
                    if unit_nw:
                        # alternate engines by m so neither scalar nor
                        # vector backlogs behind the 3.4us/m tensor pace
                        if m % 2 == 0:
                            nc.scalar.activation(
                                yn[:, hs],
                                y_all[:, m, hs],
                                mybir.ActivationFunctionType.Identity,
                                bias=0.0,
                                scale=rstd[:],
                            )
                        else:
                            nc.vector.tensor_scalar_mul(
                                yn[:, hs], y_all[:, m, hs], rstd[:]
                            )
                    else:
                        nc.vector.scalar_tensor_tensor(
                            yn[:, hs],
                            y_all[:, m, hs],
                            rstd[:],
                            nw_sb[:, hs],
                            op0=mybir.AluOpType.mult,
                            op1=mybir.AluOpType.mult,
                        )
                    nc.sync.dma_start(Y[m * P : (m + 1) * P, hs], yn[:, hs])

            pss = [psp.tile([P, 512], F32, tag=f"ps{m}", name=f"ps{m}")
                   for m in range(M)]

            # PE warm-up during the DMA lead-in (see build_l1)
            warm = consts.tile([P, 64], BF)
            nc.vector.memset(warm[:], 1.0)
            for _ in range(48):
                nc.tensor.matmul(pss[0][0:64, 0:64], lhsT=warm[:, 0:64],
                                 rhs=warm[:], start=True, stop=True)

            for n in range(NC4):
                n0 = n * 512
                xin_ts = []
                for m in range(M):
                    xt_ = sqp.tile([P, 512], BF, tag=f"xin{m}", name=f"xin{m}")
                    nc.sync.dma_start(xt_[:], XINr[:, m, n0 : n0 + 512])
                    xin_ts.append(xt_)
                if n + 2 < NC4:
                    wo_n2 = []
                    for b in range(KB):
                        w_b = wop.tile([P, 4, 512], BF, tag=f"wo{b}", name=f"wo{b}")
                        nc.sync.dma_start(w_b[:], WO[b, n + 2])
                        wo_n2.append(w_b)
                if n == 0:
                    # k-outer for the first chunk: consumes each (wo, ct)
                    # bundle over 8 matmuls, pacing the k-loop to the DMA
                    # stream instead of stalling m=0 on the full 6MB.
                    for k in range(K):
                        for m in range(M):
                            nc.tensor.matmul(
                                pss[m][:],
                                lhsT=ct_bs[k // 4][:, k % 4, m * P : (m + 1) * P],
                                rhs=wo_cur[k // 4][:, k % 4, :],
                                start=(k == 0),
                                stop=(k == K - 1),
                            )
                    for m in range(M):
                        y_slice = y_all[:, m, n0 : n0 + 512]
                        nc.vector.tensor_add(y_slice, xin_ts[m][:], pss[m][:])
                        stats(m, n, y_slice)
                else:
                    # m-outer, k-inner: 16 consecutive matmuls accumulate
                    # into one PSUM bank before it's read (avoids psum-queue
                    # depth-cycling micro-idles).
                    for m in range(M):
                        for k in range(K):
                            nc.tensor.matmul(
                                pss[m][:],
                                lhsT=ct_bs[k // 4][:, k % 4, m * P : (m + 1) * P],
                                rhs=wo_cur[k // 4][:, k % 4, :],
                                start=(k == 0),
                                stop=(k == K - 1),
                            )
                        y_slice = y_all[:, m, n0 : n0 + 512]
                        nc.vector.tensor_add(y_slice, xin_ts[m][:], pss[m][:])
                        stats(m, n, y_slice)
                        if n + 1 == NC4:
                            # chain each m's epilogue behind its own k-loop
                            # so only m=M-1's trails the final matmul
                            epilogue(m)
                if n + 1 < NC4:
                    wo_cur = wo_nxt
                    if n + 2 < NC4:
                        wo_nxt = wo_n2
    nc.compile()
    return nc


def host_dispatch(xf, Wr, br):
    """Router + top-2 + softmax gates + expert grouping. Returns dispatch info."""
    T, D = xf.shape
    E = Wr.shape[1]
    logits = xf @ Wr + br
    i1 = np.argmax(logits, axis=1)
    l2 = logits.copy()
    l2[np.arange(T), i1] = -np.inf
    i2 = np.argmax(l2, axis=1)
    v1 = logits[np.arange(T), i1]
    v2 = logits[np.arange(T), i2]
    e2 = np.exp(v2 - v1)
    g1 = (1.0 / (1.0 + e2)).astype(np.float32)
    g2 = (e2 / (1.0 + e2)).astype(np.float32)

    # flat slots (t,s) grouped by expert, stable by (token, slot)
    ee = np.stack([i1, i2], 1).ravel()          # [2T]
    gg = np.stack([g1, g2], 1).ravel()
    tt = np.repeat(np.arange(T), 2)
    order = np.argsort(ee, kind="stable")
    counts = np.bincount(ee, minlength=E)
    starts = np.concatenate([[0], np.cumsum(counts)[:-1]])
    rank = np.empty(2 * T, np.int64)
    rank[order] = np.arange(2 * T)
    pos = rank - starts[ee]                      # position within expert's list
    return dict(
        e1=i1, e2=i2, counts=counts, order=order, starts=starts,
        pos=pos.reshape(T, 2), tok=tt, gate=gg, g1=g1, g2=g2,
    )


def bcap_for(counts):
    return int(np.ceil(max(int(counts.max()), 512) / 32) * 32)


def prep_l1_inputs(xf, d, We, be):
    """Per-expert L1 inputs: gathered+pretiled bf16 XT, bf16 W, fp32 be.

    XT0 holds the first 384 tokens (small lead-in chunk); XT holds the
    remainder re-tiled into 512-column slabs.
    """
    T, D = xf.shape
    E = We.shape[0]
    K = D // P
    F = D // P
    counts = d["counts"]
    Bcap = bcap_for(counts)
    Crest = len(_chunks(Bcap - 384, 512))
    Rpad = Crest * 512
    We_f = np.asarray(We, np.float32)
    be_f = np.asarray(be, np.float32)
    in1 = []
    for e in range(E):
        sel = d["order"][d["starts"][e] : d["starts"][e] + counts[e]]
        Xg = np.zeros((384 + Rpad, D), np.float32)
        Xg[: counts[e]] = xf[d["tok"][sel]]
        # [P, K, n]: contiguous per-partition DMA runs
        XT0 = np.ascontiguousarray(
            Xg[:384].reshape(1, 384, K, P).transpose(0, 3, 2, 1)[0]
        ).astype(BF16)
        XT_T = np.ascontiguousarray(
            Xg[384:].reshape(Crest, 512, K, P).transpose(0, 3, 2, 1)
        ).astype(BF16)
        W_T = np.ascontiguousarray(
            We_f[e].reshape(K, P, F, P).transpose(2, 1, 0, 3)
        ).astype(BF16)
        in1.append({"XT0": XT0, "XT": XT_T, "W": W_T, "BE": be_f[e]})
    return in1, Bcap


def prep_l2_inputs(xf, d, H, Wo, bo, norm_w):
    """Per-core L2 inputs. CT = g1*A + g2*B combined on host (fp32 math,
    one bf16 downcast); XIN = x + bo in fp32."""
    T, D = xf.shape
    TPC = T // NCORE
    KB = D // P // 4
    NC4 = D // 512
    # pretile Wo into contiguous (k-bundle, n-chunk) blocks
    Wo_b = np.ascontiguousarray(
        np.asarray(Wo, np.float32)
        .reshape(KB, 4, P, NC4, 512)
        .transpose(0, 3, 2, 1, 4)
    ).astype(BF16)
    bo_f = np.asarray(bo, np.float32)
    nw_f = np.asarray(norm_w, np.float32)
    e1, e2, pos = d["e1"], d["e2"], d["pos"]
    g1, g2 = d["g1"], d["g2"]
    in2 = []
    for c in range(NCORE):
        tl = np.arange(c * TPC, (c + 1) * TPC)
        CTf = np.empty((D, TPC), np.float32)
        BTf = np.empty((D, TPC), np.float32)
        for e in range(H.shape[0]):
            s1 = e1[tl] == e
            if s1.any():
                CTf[:, s1] = H[e][:, pos[tl[s1], 0]]
            s2 = e2[tl] == e
            if s2.any():
                BTf[:, s2] = H[e][:, pos[tl[s2], 1]]
        CTf = CTf * g1[tl][None, :] + BTf * g2[tl][None, :]
        CTt = np.ascontiguousarray(
            CTf.reshape(KB, 4, P, TPC).transpose(0, 2, 1, 3)
        ).astype(BF16)
        XIN = (xf[tl] + bo_f[None, :]).astype(BF16)
        in2.append({"CT": CTt, "XIN": XIN, "WO": Wo_b, "NW": nw_f})
    return in2


# ----------------------------------------------------------------------------
# Harness entry point: full (unsharded) inputs -> full output.
# ----------------------------------------------------------------------------
_L1_CACHE = {}
_L2_CACHE = {}


def kernel(x, Wr, br, We, be, Wo, bo, norm_w):
    B, S, D = x.shape
    E = We.shape[0]
    T = B * S
    TPC = T // NCORE
    xf = np.ascontiguousarray(np.asarray(x, np.float32).reshape(T, D))
    d = host_dispatch(xf, np.asarray(Wr, np.float32), np.asarray(br, np.float32))

    in1, Bcap = prep_l1_inputs(xf, d, We, be)
    if (D, Bcap) not in _L1_CACHE:
        _L1_CACHE[(D, Bcap)] = build_l1(D, Bcap)
    r1 = run_bass_kernel_spmd(_L1_CACHE[(D, Bcap)], in1, list(range(NCORE)))
    H = np.stack([r1.results[e]["H"] for e in range(E)])

    in2 = prep_l2_inputs(xf, d, H, Wo, bo, norm_w)
    unit_nw = bool(np.all(np.asarray(norm_w, np.float32) == 1.0))
    if (D, TPC, unit_nw) not in _L2_CACHE:
        _L2_CACHE[(D, TPC, unit_nw)] = build_l2(D, TPC, unit_nw)
    r2 = run_bass_kernel_spmd(_L2_CACHE[(D, TPC, unit_nw)], in2, list(range(NCORE)))
    Y = np.concatenate([r2.results[c]["Y"] for c in range(NCORE)], axis=0)
    return Y.reshape(B, S, D).astype(np.asarray(x).dtype)


# revision 57
# speedup vs baseline: 1.0234x; 1.0014x over previous
# MoE EnhancedGatedFusion kernel for 8x TRN2 NeuronCores (expert-parallel).
#
# Decomposition:
#   host : router logits -> top2 -> softmax gates -> dispatch by expert
#   L1   : per-core (expert e): H_T[d_out, n] = silu(We[e].T-contract @ XT + be[e])
#          (ungated; bf16 matmul operands, fp32 PSUM + fp32 H output)
#   host : combine CT = g1*A + g2*B during the token un-shuffle (the
#          "all-to-all return" glue), downcast once to bf16.
#   L2   : per-core (1024 tokens): out = CT.T @ Wo; y = XIN + out (XIN =
#          x_shard + bo folded on host, fp32); RMS-norm * norm_w.
#
# Matmul operands are bf16: same 1 cycle/row PE rate as fp32r but half the
# HBM/SBUF traffic, and FWL (fast weight load) kicks in for non-fp32 dtypes
# so LDWEIGHTS hides under the 64-deep PE reorder window.
import sys
import types

sys.path.insert(0, "/opt/trn_rl_repo")

import numpy as np
import ml_dtypes

BF16 = np.dtype(ml_dtypes.bfloat16)


def _install_ntff_hook():
    # antenv.axon_hooks is missing in this image; shim it so
    # run_bass_kernel_spmd(trace=True) can drive NTFF profiling.
    if "antenv.axon_hooks" in sys.modules:
        return
    try:
        from trn_agent_boot.trn_boot import _ntff_profile_via_ctypes

        hook = _ntff_profile_via_ctypes("/opt/axon/libaxon_pjrt.so")
    except Exception:
        hook = None
    mod = types.ModuleType("antenv.axon_hooks")
    mod.get_axon_ntff_profile_hook = lambda: hook
    mod.set_axon_ntff_profile_hook = lambda h: None
    sys.modules["antenv.axon_hooks"] = mod


_install_ntff_hook()

import concourse.bacc as bacc
import concourse.bass as bass
import concourse.tile as tile
from concourse import mybir
from concourse.bass_utils import run_bass_kernel_spmd

F32 = mybir.dt.float32
BF = mybir.dt.bfloat16
P = 128
NCORE = 8


def _chunks(total, size):
    out = []
    o = 0
    while o < total:
        out.append((o, min(size, total - o)))
        o += size
    return out


def build_l1(D, Bcap):
    """Per-core expert FFN: H[d_out, n] = silu(sum_k W[k,d_out]*XT[k,n] + be[d_out]).

    XT_T is host-pretiled bf16 [C, P, K, 512] (zero-padded ragged tail) and
    W_T is bf16 [F, P, K, P]; W is fully SBUF-resident (8.4MB), XT streams
    through once.  H output is fp32.

    The first 512-slab is processed as two 256-col sub-chunks so the
    lead-in only gates on 1MB of XT + the first W tile; remaining W tiles
    stream behind while the f-loop burns through the small chunk.
    """
    K = D // P          # k-tiles
    F = D // P          # feat (d_out) tiles
    chunks = [(0, 384)] + [(384 + o, n) for o, n in _chunks(Bcap - 384, 512)]
    Crest = len(chunks) - 1
    nc = bacc.Bacc("TRN2", target_bir_lowering=False, debug=False)
    XT0 = nc.dram_tensor("XT0", [P, K, 384], BF, kind="ExternalInput")
    XT = nc.dram_tensor("XT", [Crest, P, K, 512], BF, kind="ExternalInput")
    W = nc.dram_tensor("W", [F, P, K, P], BF, kind="ExternalInput")
    BE = nc.dram_tensor("BE", [D], F32, kind="ExternalInput")
    # H is bf16: DMA runs packet-overhead-limited at ~155GB/s effective,
    # so halving the 17.8MB H write materially unloads the engines
    H = nc.dram_tensor("H", [D, Bcap], BF, kind="ExternalOutput")

    Hr = H[:, :].rearrange("(fo p) n -> p fo n", p=P)

    with tile.TileContext(nc) as tc:
        with (
            tc.tile_pool(name="consts", bufs=1) as consts,
            tc.tile_pool(name="xt", bufs=3) as xtp,
            tc.tile_pool(name="wf", bufs=1) as wfp,
            tc.tile_pool(name="hout", bufs=4) as hp,
            tc.tile_pool(name="ps", bufs=4, space="PSUM") as psp,
        ):
            # DMA issue order tuned for the lead-in: w0, be, first 256 cols
            # of XT, then remaining W tiles (second sub-chunk's XT slotted
            # midway so it lands before the f-loop reaches it).
            # PE warm-up: ~48 tiny matmuls during the DMA lead-in keep the
            # HAM activity window busy so real matmuls start at 2.4GHz.
            warm = consts.tile([P, 64], BF)
            nc.vector.memset(warm[:], 1.0)
            wps = psp.tile([P, 512], F32, tag="ps", name="ps")
            for _ in range(48):
                nc.tensor.matmul(wps[0:64, 0:64], lhsT=warm[:, 0:64],
                                 rhs=warm[:], start=True, stop=True)

            w_tiles = [None] * F
            w_tiles[0] = wfp.tile([P, K, P], BF, tag="wf0", name="wf0")
            nc.sync.dma_start(w_tiles[0][:, 0 : K // 2, :], W[0, :, 0 : K // 2, :])
            xt_tiles = {}
            xt_tiles[0] = xtp.tile([P, K, 512], BF, tag="xt", name="xt")
            nc.sync.dma_start(xt_tiles[0][:, 0 : K // 2, 0:384],
                              XT0[:, 0 : K // 2, :])
            nc.sync.dma_start(w_tiles[0][:, K // 2 : K, :], W[0, :, K // 2 : K, :])
            nc.sync.dma_start(xt_tiles[0][:, K // 2 : K, 0:384],
                              XT0[:, K // 2 : K, :])
            be_sb = consts.tile([P, F], F32)
            nc.sync.dma_start(be_sb[:], BE[:].rearrange("(f p) -> p f", p=P))
            for f in range(1, F):
                w_f = wfp.tile([P, K, P], BF, tag=f"wf{f}", name=f"wf{f}")
                nc.sync.dma_start(w_f[:], W[f])
                w_tiles[f] = w_f

            for ci, (c0, cn) in enumerate(chunks):
                if ci in xt_tiles:
                    xt_c = xt_tiles[ci]
                else:
                    xt_c = xtp.tile([P, K, 512], BF, tag="xt", name="xt")
                    nc.sync.dma_start(xt_c[:, :, :cn],
                                      XT[ci - 1, :, :, :cn])
                for f in range(F):
                    ps = psp.tile([P, 512], F32, tag="ps", name="ps")
                    for k in range(K):
                        nc.tensor.matmul(
                            ps[:, :cn],
                            lhsT=w_tiles[f][:, k, :],
                            rhs=xt_c[:, k, :cn],
                            start=(k == 0),
                            stop=(k == K - 1),
                        )
                    h_t = hp.tile([P, 512], BF, tag="h", name="h")
                    nc.scalar.activation(
                        h_t[:, :cn],
                        ps[:, :cn],
                        mybir.ActivationFunctionType.Silu,
                        bias=be_sb[:, f : f + 1],
                        scale=1.0,
                    )
                    nc.sync.dma_start(Hr[:, f, c0 : c0 + cn], h_t[:, :cn])
    nc.compile()
    return nc


def build_l2(D, TPC, unit_nw, eps=1e-6):
    """Per-core combine + output proj + residual + RMS norm.

    Y[t, j] = nw[j] * (XIN[t,j] + sum_k CT[k,t]*Wo[k,j]) / rms(t)
    CT = g1*A + g2*B (host-combined, bf16); XIN = x_shard + bo (fp32).
    Y output is bf16 (host upcasts).

    The last n-chunk runs m-outer so each m's epilogue (RMS + scale +
    store) chains behind its own k-loop and overlaps the next m's
    matmuls; only m=M-1's epilogue trails the final matmul.

    unit_nw=True specializes norm_w == 1 (scale-by-rstd runs as a scalar
    engine activation; multiplying by 1 is exact) so the vector engine
    stays under the per-m tensor budget during the epilogue phase.
    """
    K = D // P
    M = TPC // P
    NC4 = D // 512
    KB = K // 4          # k-tiles bundled per DMA
    nc = bacc.Bacc("TRN2", target_bir_lowering=False, debug=False)
    # CT/WO are host-pretiled so every bundle DMA reads long contiguous
    # per-partition runs (8KB / 4KB) — strided reads from the natural
    # [D, x] layout only sustain ~40% of DMA bandwidth and gate the lead-in.
    CT = nc.dram_tensor("CT", [KB, P, 4, TPC], BF, kind="ExternalInput")
    XIN = nc.dram_tensor("XIN", [TPC, D], BF, kind="ExternalInput")
    WO = nc.dram_tensor("WO", [KB, NC4, P, 4, 512], BF, kind="ExternalInput")
    NW = nc.dram_tensor("NW", [D], F32, kind="ExternalInput")
    Y = nc.dram_tensor("Y", [TPC, D], BF, kind="ExternalOutput")

    XINr = XIN[:, :].rearrange("(m p) d -> p m d", p=P)

    with tile.TileContext(nc) as tc:
        with (
            tc.tile_pool(name="consts", bufs=1) as consts,
            tc.tile_pool(name="ct", bufs=1) as ctp,
            tc.tile_pool(name="wo", bufs=3) as wop,
            tc.tile_pool(name="yall", bufs=1) as yallp,
            tc.tile_pool(name="sq", bufs=3) as sqp,
            tc.tile_pool(name="yn", bufs=2) as ynp,
            tc.tile_pool(name="ssm", bufs=1) as ssmp,
            tc.tile_pool(name="stat", bufs=4) as statp,
            tc.tile_pool(name="ps", bufs=1, space="PSUM") as psp,
        ):
            # Bundled DMAs (4 k-tiles each), interleaved (wo, ct) pairs so
            # the k-loop's operands arrive in consumption order; XIN and nw
            # are queued behind everything n=0/n=1 needs.
            ct_bs = []
            wo_cur = []
            for b in range(KB):
                w_b = wop.tile([P, 4, 512], BF, tag=f"wo{b}", name=f"wo{b}")
                nc.sync.dma_start(w_b[:], WO[b, 0])
                wo_cur.append(w_b)
                c_b = ctp.tile([P, 4, TPC], BF, tag=f"ct{b}", name=f"ct{b}")
                nc.sync.dma_start(c_b[:], CT[b])
                ct_bs.append(c_b)
            wo_nxt = []
            for b in range(KB):
                w_b = wop.tile([P, 4, 512], BF, tag=f"wo{b}", name=f"wo{b}")
                nc.sync.dma_start(w_b[:], WO[b, 1])
                wo_nxt.append(w_b)
            # y_all accumulator (fp32); the residual XIN streams in as
            # per-(m,n) bf16 slices added at psum-eviction time, so its
            # bytes never compete with the lead-in wo/ct stream.
            y_all = yallp.tile([P, M, D], F32)
            nw_sb = None
            if not unit_nw:
                nw_sb = consts.tile([P, D], F32)
                nwap = NW[:]
                nw_bcast = bass.AP(
                    tensor=nwap.tensor, offset=nwap.offset, ap=[[0, P]] + list(nwap.ap)
                )
                nc.sync.dma_start(nw_sb[:], nw_bcast)
            eps_sb = consts.tile([P, 1], F32)
            nc.vector.memset(eps_sb[:], eps)

            ssm_t = ssmp.tile([P, M], F32)
            ss_m = [ssm_t[:, m : m + 1] for m in range(M)]

            def stats(m, n, y_slice):
                # incremental RMS stats: ss_m[m] += sum(y_slice^2)
                sq = sqp.tile([P, 512], F32, tag="sq", name="sq")
                ssp = statp.tile([P, 1], F32, tag="ssp", name="ssp")
                nc.scalar.activation(
                    sq[:],
                    y_slice,
                    mybir.ActivationFunctionType.Square,
                    accum_out=ssp[:],
                )
                if n == 0:
                    nc.vector.tensor_copy(ss_m[m], ssp[:])
                else:
                    nc.vector.tensor_add(ss_m[m], ss_m[m], ssp[:])

            def epilogue(m):
                y_m = y_all[:, m, :]
                rms = statp.tile([P, 1], F32, tag="rms", name="rms")
                nc.scalar.activation(
                    rms[:],
                    ss_m[m],
                    mybir.ActivationFunctionType.Sqrt,
                    bias=eps_sb[:],
                    scale=1.0 / D,
                )
                rstd = statp.tile([P, 1], F32, tag="rstd", name="rstd")
                nc.vector.reciprocal(rstd[:], rms[:])
                yn = ynp.tile([P, D], BF, tag="yn", name="yn")
                if unit_nw and m == M - 1:
                    # final m: its scale trails the last matmul, so run the
                    # two halves concurrently on scalar and vector
                    nc.scalar.activation(
                        yn[:, 0 : D // 2],
                        y_all[:, m, 0 : D // 2],
                        mybir.ActivationFunctionType.Identity,
                        bias=0.0,
                        scale=rstd[:],
                    )
                    nc.vector.tensor_scalar_mul(
                        yn[:, D // 2 : D], y_all[:, m, D // 2 : D], rstd[:]
                    )
                    nc.sync.dma_start(Y[m * P : (m + 1) * P, 0 : D // 2],
                                      yn[:, 0 : D // 2])
                    nc.sync.dma_start(Y[m * P : (m + 1) * P, D // 2 : D],
                                      yn[:, D // 2 : D])
                    return
                for h in range(1):
                    hs = slice(0, D)
                    if unit_nw:
                        # alternate engines by m so neither scalar nor
                        # vector backlogs behind the 3.4us/m tensor pace
                        if m % 2 == 0:
                            nc.scalar.activation(
                                yn[:, hs],
                                y_all[:, m, hs],
                                mybir.ActivationFunctionType.Identity,
                                bias=0.0,
                                scale=rstd[:],
                            )
                        else:
                            nc.vector.tensor_scalar_mul(
                                yn[:, hs], y_all[:, m, hs], rstd[:]
                            )
                    else:
                        nc.vector.scalar_tensor_tensor(
                            yn[:, hs],
                            y_all[:, m, hs],
                            rstd[:],
                            nw_sb[:, hs],
                            op0=mybir.AluOpType.mult,
                            op1=mybir.AluOpType.mult,
                        )
                    nc.sync.dma_start(Y[m * P : (m + 1) * P, hs], yn[:, hs])

            pss = [psp.tile([P, 512], F32, tag=f"ps{m}", name=f"ps{m}")
                   for m in range(M)]

            # PE warm-up during the DMA lead-in (see build_l1)
            warm = consts.tile([P, 64], BF)
            nc.vector.memset(warm[:], 1.0)
            for _ in range(48):
                nc.tensor.matmul(pss[0][0:64, 0:64], lhsT=warm[:, 0:64],
                                 rhs=warm[:], start=True, stop=True)

            for n in range(NC4):
                n0 = n * 512
                xin_ts = []
                for m in range(M):
                    xt_ = sqp.tile([P, 512], BF, tag=f"xin{m}", name=f"xin{m}")
                    nc.sync.dma_start(xt_[:], XINr[:, m, n0 : n0 + 512])
                    xin_ts.append(xt_)
                if n + 2 < NC4:
                    wo_n2 = []
                    for b in range(KB):
                        w_b = wop.tile([P, 4, 512], BF, tag=f"wo{b}", name=f"wo{b}")
                        nc.sync.dma_start(w_b[:], WO[b, n + 2])
                        wo_n2.append(w_b)
                if n == 0:
                    # k-outer for the first chunk: consumes each (wo, ct)
                    # bundle over 8 matmuls, pacing the k-loop to the DMA
                    # stream instead of stalling m=0 on the full 6MB.
                    for k in range(K):
                        for m in range(M):
                            nc.tensor.matmul(
                                pss[m][:],
                                lhsT=ct_bs[k // 4][:, k % 4, m * P : (m + 1) * P],
                                rhs=wo_cur[k // 4][:, k % 4, :],
                                start=(k == 0),
                                stop=(k == K - 1),
                            )
                    for m in range(M):
                        y_slice = y_all[:, m, n0 : n0 + 512]
                        nc.vector.tensor_add(y_slice, xin_ts[m][:], pss[m][:])
                        stats(m, n, y_slice)
                else:
                    # m-outer, k-inner: 16 consecutive matmuls accumulate
                    # into one PSUM bank before it's read (avoids psum-queue
                    # depth-cycling micro-idles).
                    for m in range(M):
                        for k in range(K):
                            nc.tensor.matmul(
                                pss[m][:],
                                lhsT=ct_bs[k // 4][:, k % 4, m * P : (m + 1) * P],
                                rhs=wo_cur[k // 4][:, k % 4, :],
                                start=(k == 0),
                                stop=(k == K - 1),
                            )
                        y_slice = y_all[:, m, n0 : n0 + 512]
                        nc.vector.tensor_add(y_slice, xin_ts[m][:], pss[m][:])
                        stats(m, n, y_slice)
                        if n + 1 == NC4:
                            # chain each m's epilogue behind its own k-loop
                            # so only m=M-1's trails the final matmul
                            epilogue(m)
                if n + 1 < NC4:
                    wo_cur = wo_nxt
                    if n + 2 < NC4:
                        wo_nxt = wo_n2
    nc.compile()
    return nc


def host_dispatch(xf, Wr, br):
    """Router + top-2 + softmax gates + expert grouping. Returns dispatch info."""
    T, D = xf.shape
    E = Wr.shape[1]
    logits = xf @ Wr + br
    i1 = np.argmax(logits, axis=1)
    l2 = logits.copy()
    l2[np.arange(T), i1] = -np.inf
    i2 = np.argmax(l2, axis=1)
    v1 = logits[np.arange(T), i1]
    v2 = logits[np.arange(T), i2]
    e2 = np.exp(v2 - v1)
    g1 = (1.0 / (1.0 + e2)).astype(np.float32)
    g2 = (e2 / (1.0 + e2)).astype(np.float32)

    # flat slots (t,s) grouped by expert, stable by (token, slot)
    ee = np.stack([i1, i2], 1).ravel()          # [2T]
    gg = np.stack([g1, g2], 1).ravel()
    tt = np.repeat(np.arange(T), 2)
    order = np.argsort(ee, kind="stable")
    counts = np.bincount(ee, minlength=E)
    starts = np.concatenate([[0], np.cumsum(counts)[:-1]])
    rank = np.empty(2 * T, np.int64)
    rank[order] = np.arange(2 * T)
    pos = rank - starts[ee]                      # position within expert's list
    return dict(
        e1=i1, e2=i2, counts=counts, order=order, starts=starts,
        pos=pos.reshape(T, 2), tok=tt, gate=gg, g1=g1, g2=g2,
    )


def bcap_for(counts):
    return int(np.ceil(max(int(counts.max()), 512) / 32) * 32)


def prep_l1_inputs(xf, d, We, be):
    """Per-expert L1 inputs: gathered+pretiled bf16 XT, bf16 W, fp32 be.

    XT0 holds the first 384 tokens (small lead-in chunk); XT holds the
    remainder re-tiled into 512-column slabs.
    """
    T, D = xf.shape
    E = We.shape[0]
    K = D // P
    F = D // P
    counts = d["counts"]
    Bcap = bcap_for(counts)
    Crest = len(_chunks(Bcap - 384, 512))
    Rpad = Crest * 512
    We_f = np.asarray(We, np.float32)
    be_f = np.asarray(be, np.float32)
    in1 = []
    for e in range(E):
        sel = d["order"][d["starts"][e] : d["starts"][e] + counts[e]]
        Xg = np.zeros((384 + Rpad, D), np.float32)
        Xg[: counts[e]] = xf[d["tok"][sel]]
        # [P, K, n]: contiguous per-partition DMA runs
        XT0 = np.ascontiguousarray(
            Xg[:384].reshape(1, 384, K, P).transpose(0, 3, 2, 1)[0]
        ).astype(BF16)
        XT_T = np.ascontiguousarray(
            Xg[384:].reshape(Crest, 512, K, P).transpose(0, 3, 2, 1)
        ).astype(BF16)
        W_T = np.ascontiguousarray(
            We_f[e].reshape(K, P, F, P).transpose(2, 1, 0, 3)
        ).astype(BF16)
        in1.append({"XT0": XT0, "XT": XT_T, "W": W_T, "BE": be_f[e]})
    return in1, Bcap


def prep_l2_inputs(xf, d, H, Wo, bo, norm_w):
    """Per-core L2 inputs. CT = g1*A + g2*B combined on host (fp32 math,
    one bf16 downcast); XIN = x + bo in fp32."""
    T, D = xf.shape
    TPC = T // NCORE
    KB = D // P // 4
    NC4 = D // 512
    # pretile Wo into contiguous (k-bundle, n-chunk) blocks
    Wo_b = np.ascontiguousarray(
        np.asarray(Wo, np.float32)
        .reshape(KB, 4, P, NC4, 512)
        .transpose(0, 3, 2, 1, 4)
    ).astype(BF16)
    bo_f = np.asarray(bo, np.float32)
    nw_f = np.asarray(norm_w, np.float32)
    e1, e2, pos = d["e1"], d["e2"], d["pos"]
    g1, g2 = d["g1"], d["g2"]
    in2 = []
    for c in range(NCORE):
        tl = np.arange(c * TPC, (c + 1) * TPC)
        CTf = np.empty((D, TPC), np.float32)
        BTf = np.empty((D, TPC), np.float32)
        for e in range(H.shape[0]):
            s1 = e1[tl] == e
            if s1.any():
                CTf[:, s1] = H[e][:, pos[tl[s1], 0]]
            s2 = e2[tl] == e
            if s2.any():
                BTf[:, s2] = H[e][:, pos[tl[s2], 1]]
        CTf = CTf * g1[tl][None, :] + BTf * g2[tl][None, :]
        CTt = np.ascontiguousarray(
            CTf.reshape(KB, 4, P, TPC).transpose(0, 2, 1, 3)
        ).astype(BF16)
        XIN = (xf[tl] + bo_f[None, :]).astype(BF16)
        in2.append({"CT": CTt, "XIN": XIN, "WO": Wo_b, "NW": nw_f})
    return in2


# ----------------------------------------------------------------------------
# Harness entry point: full (unsharded) inputs -> full output.
# ----------------------------------------------------------------------------
_L1_CACHE = {}
_L2_CACHE = {}


def kernel(x, Wr, br, We, be, Wo, bo, norm_w):
    B, S, D = x.shape
    E = We.shape[0]
    T = B * S
    TPC = T // NCORE
    xf = np.ascontiguousarray(np.asarray(x, np.float32).reshape(T, D))
    d = host_dispatch(xf, np.asarray(Wr, np.float32), np.asarray(br, np.float32))

    in1, Bcap = prep_l1_inputs(xf, d, We, be)
    if (D, Bcap) not in _L1_CACHE:
        _L1_CACHE[(D, Bcap)] = build_l1(D, Bcap)
    r1 = run_bass_kernel_spmd(_L1_CACHE[(D, Bcap)], in1, list(range(NCORE)))
    H = np.stack([r1.results[e]["H"] for e in range(E)])

    in2 = prep_l2_inputs(xf, d, H, Wo, bo, norm_w)
    unit_nw = bool(np.all(np.asarray(norm_w, np.float32) == 1.0))
    if (D, TPC, unit_nw) not in _L2_CACHE:
        _L2_CACHE[(D, TPC, unit_nw)] = build_l2(D, TPC, unit_nw)
    r2 = run_bass_kernel_spmd(_L2_CACHE[(D, TPC, unit_nw)], in2, list(range(NCORE)))
    Y = np.concatenate([r2.results[c]["Y"] for c in range(NCORE)], axis=0)
    return Y.reshape(B, S, D).astype(np.asarray(x).dtype)


# revision 58
# speedup vs baseline: 1.0335x; 1.0098x over previous
# MoE EnhancedGatedFusion kernel for 8x TRN2 NeuronCores (expert-parallel).
#
# Decomposition:
#   host : router logits -> top2 -> softmax gates -> dispatch by expert
#   L1   : per-core (expert e): H_T[d_out, n] = silu(We[e].T-contract @ XT + be[e])
#          (ungated; bf16 matmul operands, fp32 PSUM + fp32 H output)
#   host : combine CT = g1*A + g2*B during the token un-shuffle (the
#          "all-to-all return" glue), downcast once to bf16.
#   L2   : per-core (1024 tokens): out = CT.T @ Wo; y = XIN + out (XIN =
#          x_shard + bo folded on host, fp32); RMS-norm * norm_w.
#
# Matmul operands are bf16: same 1 cycle/row PE rate as fp32r but half the
# HBM/SBUF traffic, and FWL (fast weight load) kicks in for non-fp32 dtypes
# so LDWEIGHTS hides under the 64-deep PE reorder window.
import sys
import types

sys.path.insert(0, "/opt/trn_rl_repo")

import numpy as np
import ml_dtypes

BF16 = np.dtype(ml_dtypes.bfloat16)


def _install_ntff_hook():
    # antenv.axon_hooks is missing in this image; shim it so
    # run_bass_kernel_spmd(trace=True) can drive NTFF profiling.
    if "antenv.axon_hooks" in sys.modules:
        return
    try:
        from trn_agent_boot.trn_boot import _ntff_profile_via_ctypes

        hook = _ntff_profile_via_ctypes("/opt/axon/libaxon_pjrt.so")
    except Exception:
        hook = None
    mod = types.ModuleType("antenv.axon_hooks")
    mod.get_axon_ntff_profile_hook = lambda: hook
    mod.set_axon_ntff_profile_hook = lambda h: None
    sys.modules["antenv.axon_hooks"] = mod


_install_ntff_hook()

import concourse.bacc as bacc
import concourse.bass as bass
import concourse.tile as tile
from concourse import mybir
from concourse.bass_utils import run_bass_kernel_spmd

F32 = mybir.dt.float32
BF = mybir.dt.bfloat16
P = 128
NCORE = 8


def _chunks(total, size):
    out = []
    o = 0
    while o < total:
        out.append((o, min(size, total - o)))
        o += size
    return out


def build_l1(D, Bcap):
    """Per-core expert FFN: H[d_out, n] = silu(sum_k W[k,d_out]*XT[k,n] + be[d_out]).

    XT_T is host-pretiled bf16 [C, P, K, 512] (zero-padded ragged tail) and
    W_T is bf16 [F, P, K, P]; W is fully SBUF-resident (8.4MB), XT streams
    through once.  H output is fp32.

    The first 512-slab is processed as two 256-col sub-chunks so the
    lead-in only gates on 1MB of XT + the first W tile; remaining W tiles
    stream behind while the f-loop burns through the small chunk.
    """
    K = D // P          # k-tiles
    F = D // P          # feat (d_out) tiles
    chunks = [(0, 384)] + [(384 + o, n) for o, n in _chunks(Bcap - 384, 512)]
    Crest = len(chunks) - 1
    nc = bacc.Bacc("TRN2", target_bir_lowering=False, debug=False)
    XT0 = nc.dram_tensor("XT0", [P, K, 384], BF, kind="ExternalInput")
    XT = nc.dram_tensor("XT", [Crest, P, K, 512], BF, kind="ExternalInput")
    W = nc.dram_tensor("W", [F, P, K, P], BF, kind="ExternalInput")
    BE = nc.dram_tensor("BE", [D], F32, kind="ExternalInput")
    # H is bf16: DMA runs packet-overhead-limited at ~155GB/s effective,
    # so halving the 17.8MB H write materially unloads the engines
    H = nc.dram_tensor("H", [D, Bcap], BF, kind="ExternalOutput")

    Hr = H[:, :].rearrange("(fo p) n -> p fo n", p=P)

    with tile.TileContext(nc) as tc:
        with (
            tc.tile_pool(name="consts", bufs=1) as consts,
            tc.tile_pool(name="xt", bufs=3) as xtp,
            tc.tile_pool(name="wf", bufs=1) as wfp,
            tc.tile_pool(name="hout", bufs=4) as hp,
            tc.tile_pool(name="ps", bufs=4, space="PSUM") as psp,
        ):
            # DMA issue order tuned for the lead-in: w0, be, first 256 cols
            # of XT, then remaining W tiles (second sub-chunk's XT slotted
            # midway so it lands before the f-loop reaches it).
            # PE warm-up: ~48 tiny matmuls during the DMA lead-in keep the
            # HAM activity window busy so real matmuls start at 2.4GHz.
            warm = consts.tile([P, 64], BF)
            nc.vector.memset(warm[:], 1.0)
            wps = psp.tile([P, 512], F32, tag="ps", name="ps")
            for _ in range(48):
                nc.tensor.matmul(wps[0:64, 0:64], lhsT=warm[:, 0:64],
                                 rhs=warm[:], start=True, stop=True)

            w_tiles = [None] * F
            w_tiles[0] = wfp.tile([P, K, P], BF, tag="wf0", name="wf0")
            nc.sync.dma_start(w_tiles[0][:, 0 : K // 2, :], W[0, :, 0 : K // 2, :])
            xt_tiles = {}
            xt_tiles[0] = xtp.tile([P, K, 512], BF, tag="xt", name="xt")
            nc.sync.dma_start(xt_tiles[0][:, 0 : K // 2, 0:384],
                              XT0[:, 0 : K // 2, :])
            nc.sync.dma_start(w_tiles[0][:, K // 2 : K, :], W[0, :, K // 2 : K, :])
            nc.sync.dma_start(xt_tiles[0][:, K // 2 : K, 0:384],
                              XT0[:, K // 2 : K, :])
            be_sb = consts.tile([P, F], F32)
            nc.sync.dma_start(be_sb[:], BE[:].rearrange("(f p) -> p f", p=P))
            for f in range(1, F):
                w_f = wfp.tile([P, K, P], BF, tag=f"wf{f}", name=f"wf{f}")
                nc.sync.dma_start(w_f[:], W[f])
                w_tiles[f] = w_f

            for ci, (c0, cn) in enumerate(chunks):
                if ci in xt_tiles:
                    xt_c = xt_tiles[ci]
                else:
                    xt_c = xtp.tile([P, K, 512], BF, tag="xt", name="xt")
                    nc.sync.dma_start(xt_c[:, :, :cn],
                                      XT[ci - 1, :, :, :cn])
                for f in range(F):
                    ps = psp.tile([P, 512], F32, tag="ps", name="ps")
                    for k in range(K):
                        nc.tensor.matmul(
                            ps[:, :cn],
                            lhsT=w_tiles[f][:, k, :],
                            rhs=xt_c[:, k, :cn],
                            start=(k == 0),
                            stop=(k == K - 1),
                        )
                    h_t = hp.tile([P, 512], BF, tag="h", name="h")
                    nc.scalar.activation(
                        h_t[:, :cn],
                        ps[:, :cn],
                        mybir.ActivationFunctionType.Silu,
                        bias=be_sb[:, f : f + 1],
                        scale=1.0,
                    )
                    nc.sync.dma_start(Hr[:, f, c0 : c0 + cn], h_t[:, :cn])
    nc.compile()
    return nc


def build_l2(D, TPC, unit_nw, eps=1e-6):
    """Per-core combine + output proj + residual + RMS norm.

    Y[t, j] = nw[j] * (XIN[t,j] + sum_k CT[k,t]*Wo[k,j]) / rms(t)
    CT = g1*A + g2*B (host-combined, bf16); XIN = x_shard + bo (fp32).
    Y output is bf16 (host upcasts).

    The last n-chunk runs m-outer so each m's epilogue (RMS + scale +
    store) chains behind its own k-loop and overlaps the next m's
    matmuls; only m=M-1's epilogue trails the final matmul.

    unit_nw=True specializes norm_w == 1 (scale-by-rstd runs as a scalar
    engine activation; multiplying by 1 is exact) so the vector engine
    stays under the per-m tensor budget during the epilogue phase.
    """
    K = D // P
    M = TPC // P
    NC4 = D // 512
    KB = K // 4          # k-tiles bundled per DMA
    nc = bacc.Bacc("TRN2", target_bir_lowering=False, debug=False)
    # CT/WO are host-pretiled so every bundle DMA reads long contiguous
    # per-partition runs (8KB / 4KB) — strided reads from the natural
    # [D, x] layout only sustain ~40% of DMA bandwidth and gate the lead-in.
    CT = nc.dram_tensor("CT", [KB, P, 4, TPC], BF, kind="ExternalInput")
    XIN = nc.dram_tensor("XIN", [TPC, D], BF, kind="ExternalInput")
    WO = nc.dram_tensor("WO", [KB, NC4, P, 4, 512], BF, kind="ExternalInput")
    NW = nc.dram_tensor("NW", [D], F32, kind="ExternalInput")
    Y = nc.dram_tensor("Y", [TPC, D], BF, kind="ExternalOutput")

    XINr = XIN[:, :].rearrange("(m p) d -> p m d", p=P)

    with tile.TileContext(nc) as tc:
        with (
            tc.tile_pool(name="consts", bufs=1) as consts,
            tc.tile_pool(name="ct", bufs=1) as ctp,
            tc.tile_pool(name="wo", bufs=3) as wop,
            tc.tile_pool(name="yall", bufs=1) as yallp,
            tc.tile_pool(name="sq", bufs=3) as sqp,
            tc.tile_pool(name="yn", bufs=2) as ynp,
            tc.tile_pool(name="ssm", bufs=1) as ssmp,
            tc.tile_pool(name="stat", bufs=4) as statp,
            tc.tile_pool(name="ps", bufs=1, space="PSUM") as psp,
        ):
            # Bundled DMAs (4 k-tiles each), interleaved (wo, ct) pairs so
            # the k-loop's operands arrive in consumption order; XIN and nw
            # are queued behind everything n=0/n=1 needs.
            ct_bs = []
            wo_cur = []
            for b in range(KB):
                w_b = wop.tile([P, 4, 512], BF, tag=f"wo{b}", name=f"wo{b}")
                c_b = ctp.tile([P, 4, TPC], BF, tag=f"ct{b}", name=f"ct{b}")
                if b == 0:
                    # split the first bundle per k-tile: the n=0 k-loop's
                    # first matmuls start after ~0.4MB instead of 1.5MB
                    for kk in range(4):
                        nc.sync.dma_start(w_b[:, kk, :], WO[0, 0, :, kk, :])
                        nc.sync.dma_start(c_b[:, kk, :], CT[0, :, kk, :])
                else:
                    nc.sync.dma_start(w_b[:], WO[b, 0])
                    nc.sync.dma_start(c_b[:], CT[b])
                wo_cur.append(w_b)
                ct_bs.append(c_b)
            wo_nxt = []
            for b in range(KB):
                w_b = wop.tile([P, 4, 512], BF, tag=f"wo{b}", name=f"wo{b}")
                nc.sync.dma_start(w_b[:], WO[b, 1])
                wo_nxt.append(w_b)
            # y_all accumulator (fp32); the residual XIN streams in as
            # per-(m,n) bf16 slices added at psum-eviction time, so its
            # bytes never compete with the lead-in wo/ct stream.
            y_all = yallp.tile([P, M, D], F32)
            nw_sb = None
            if not unit_nw:
                nw_sb = consts.tile([P, D], F32)
                nwap = NW[:]
                nw_bcast = bass.AP(
                    tensor=nwap.tensor, offset=nwap.offset, ap=[[0, P]] + list(nwap.ap)
                )
                nc.sync.dma_start(nw_sb[:], nw_bcast)
            eps_sb = consts.tile([P, 1], F32)
            nc.vector.memset(eps_sb[:], eps)

            ssm_t = ssmp.tile([P, M], F32)
            ss_m = [ssm_t[:, m : m + 1] for m in range(M)]

            def stats(m, n, y_slice):
                # incremental RMS stats: ss_m[m] += sum(y_slice^2)
                sq = sqp.tile([P, 512], F32, tag="sq", name="sq")
                ssp = statp.tile([P, 1], F32, tag="ssp", name="ssp")
                nc.scalar.activation(
                    sq[:],
                    y_slice,
                    mybir.ActivationFunctionType.Square,
                    accum_out=ssp[:],
                )
                if n == 0:
                    nc.vector.tensor_copy(ss_m[m], ssp[:])
                else:
                    nc.vector.tensor_add(ss_m[m], ss_m[m], ssp[:])

            def epilogue(m):
                y_m = y_all[:, m, :]
                rms = statp.tile([P, 1], F32, tag="rms", name="rms")
                nc.scalar.activation(
                    rms[:],
                    ss_m[m],
                    mybir.ActivationFunctionType.Sqrt,
                    bias=eps_sb[:],
                    scale=1.0 / D,
                )
                rstd = statp.tile([P, 1], F32, tag="rstd", name="rstd")
                nc.vector.reciprocal(rstd[:], rms[:])
                yn = ynp.tile([P, D], BF, tag="yn", name="yn")
                if unit_nw and m == M - 1:
                    # final m: its scale trails the last matmul, so run the
                    # two halves concurrently on scalar and vector
                    nc.scalar.activation(
                        yn[:, 0 : D // 2],
                        y_all[:, m, 0 : D // 2],
                        mybir.ActivationFunctionType.Identity,
                        bias=0.0,
                        scale=rstd[:],
                    )
                    nc.vector.tensor_scalar_mul(
                        yn[:, D // 2 : D], y_all[:, m, D // 2 : D], rstd[:]
                    )
                    nc.sync.dma_start(Y[m * P : (m + 1) * P, 0 : D // 2],
                                      yn[:, 0 : D // 2])
                    nc.sync.dma_start(Y[m * P : (m + 1) * P, D // 2 : D],
                                      yn[:, D // 2 : D])
                    return
                for h in range(1):
                    hs = slice(0, D)
                    if unit_nw:
                        # alternate engines by m so neither scalar nor
                        # vector backlogs behind the 3.4us/m tensor pace
                        if m % 2 == 0:
                            nc.scalar.activation(
                                yn[:, hs],
                                y_all[:, m, hs],
                                mybir.ActivationFunctionType.Identity,
                                bias=0.0,
                                scale=rstd[:],
                            )
                        else:
                            nc.vector.tensor_scalar_mul(
                                yn[:, hs], y_all[:, m, hs], rstd[:]
                            )
                    else:
                        nc.vector.scalar_tensor_tensor(
                            yn[:, hs],
                            y_all[:, m, hs],
                            rstd[:],
                            nw_sb[:, hs],
                            op0=mybir.AluOpType.mult,
                            op1=mybir.AluOpType.mult,
                        )
                    nc.sync.dma_start(Y[m * P : (m + 1) * P, hs], yn[:, hs])

            pss = [psp.tile([P, 512], F32, tag=f"ps{m}", name=f"ps{m}")
                   for m in range(M)]

            # PE warm-up during the DMA lead-in (see build_l1)
            warm = consts.tile([P, 64], BF)
            nc.vector.memset(warm[:], 1.0)
            for _ in range(48):
                nc.tensor.matmul(pss[0][0:64, 0:64], lhsT=warm[:, 0:64],
                                 rhs=warm[:], start=True, stop=True)

            for n in range(NC4):
                n0 = n * 512
                xin_ts = []
                for m in range(M):
                    xt_ = sqp.tile([P, 512], BF, tag=f"xin{m}", name=f"xin{m}")
                    nc.sync.dma_start(xt_[:], XINr[:, m, n0 : n0 + 512])
                    xin_ts.append(xt_)
                if n + 2 < NC4:
                    wo_n2 = []
                    for b in range(KB):
                        w_b = wop.tile([P, 4, 512], BF, tag=f"wo{b}", name=f"wo{b}")
                        nc.sync.dma_start(w_b[:], WO[b, n + 2])
                        wo_n2.append(w_b)
                if n == 0:
                    # k-outer for the first chunk: consumes each (wo, ct)
                    # bundle over 8 matmuls, pacing the k-loop to the DMA
                    # stream instead of stalling m=0 on the full 6MB.
                    for k in range(K):
                        for m in range(M):
                            nc.tensor.matmul(
                                pss[m][:],
                                lhsT=ct_bs[k // 4][:, k % 4, m * P : (m + 1) * P],
                                rhs=wo_cur[k // 4][:, k % 4, :],
                                start=(k == 0),
                                stop=(k == K - 1),
                            )
                    for m in range(M):
                        y_slice = y_all[:, m, n0 : n0 + 512]
                        nc.vector.tensor_add(y_slice, xin_ts[m][:], pss[m][:])
                        stats(m, n, y_slice)
                else:
                    # m-outer, k-inner: 16 consecutive matmuls accumulate
                    # into one PSUM bank before it's read (avoids psum-queue
                    # depth-cycling micro-idles).
                    for m in range(M):
                        for k in range(K):
                            nc.tensor.matmul(
                                pss[m][:],
                                lhsT=ct_bs[k // 4][:, k % 4, m * P : (m + 1) * P],
                                rhs=wo_cur[k // 4][:, k % 4, :],
                                start=(k == 0),
                                stop=(k == K - 1),
                            )
                        y_slice = y_all[:, m, n0 : n0 + 512]
                        nc.vector.tensor_add(y_slice, xin_ts[m][:], pss[m][:])
                        stats(m, n, y_slice)
                        if n + 1 == NC4:
                            # chain each m's epilogue behind its own k-loop
                            # so only m=M-1's trails the final matmul
                            epilogue(m)
                if n + 1 < NC4:
                    wo_cur = wo_nxt
                    if n + 2 < NC4:
                        wo_nxt = wo_n2
    nc.compile()
    return nc


def host_dispatch(xf, Wr, br):
    """Router + top-2 + softmax gates + expert grouping. Returns dispatch info."""
    T, D = xf.shape
    E = Wr.shape[1]
    logits = xf @ Wr + br
    i1 = np.argmax(logits, axis=1)
    l2 = logits.copy()
    l2[np.arange(T), i1] = -np.inf
    i2 = np.argmax(l2, axis=1)
    v1 = logits[np.arange(T), i1]
    v2 = logits[np.arange(T), i2]
    e2 = np.exp(v2 - v1)
    g1 = (1.0 / (1.0 + e2)).astype(np.float32)
    g2 = (e2 / (1.0 + e2)).astype(np.float32)

    # flat slots (t,s) grouped by expert, stable by (token, slot)
    ee = np.stack([i1, i2], 1).ravel()          # [2T]
    gg = np.stack([g1, g2], 1).ravel()
    tt = np.repeat(np.arange(T), 2)
    order = np.argsort(ee, kind="stable")
    counts = np.bincount(ee, minlength=E)
    starts = np.concatenate([[0], np.cumsum(counts)[:-1]])
    rank = np.empty(2 * T, np.int64)
    rank[order] = np.arange(2 * T)
    pos = rank - starts[ee]                      # position within expert's list
    return dict(
        e1=i1, e2=i2, counts=counts, order=order, starts=starts,
        pos=pos.reshape(T, 2), tok=tt, gate=gg, g1=g1, g2=g2,
    )


def bcap_for(counts):
    return int(np.ceil(max(int(counts.max()), 512) / 32) * 32)


def prep_l1_inputs(xf, d, We, be):
    """Per-expert L1 inputs: gathered+pretiled bf16 XT, bf16 W, fp32 be.

    XT0 holds the first 384 tokens (small lead-in chunk); XT holds the
    remainder re-tiled into 512-column slabs.
    """
    T, D = xf.shape
    E = We.shape[0]
    K = D // P
    F = D // P
    counts = d["counts"]
    Bcap = bcap_for(counts)
    Crest = len(_chunks(Bcap - 384, 512))
    Rpad = Crest * 512
    We_f = np.asarray(We, np.float32)
    be_f = np.asarray(be, np.float32)
    in1 = []
    for e in range(E):
        sel = d["order"][d["starts"][e] : d["starts"][e] + counts[e]]
        Xg = np.zeros((384 + Rpad, D), np.float32)
        Xg[: counts[e]] = xf[d["tok"][sel]]
        # [P, K, n]: contiguous per-partition DMA runs
        XT0 = np.ascontiguousarray(
            Xg[:384].reshape(1, 384, K, P).transpose(0, 3, 2, 1)[0]
        ).astype(BF16)
        XT_T = np.ascontiguousarray(
            Xg[384:].reshape(Crest, 512, K, P).transpose(0, 3, 2, 1)
        ).astype(BF16)
        W_T = np.ascontiguousarray(
            We_f[e].reshape(K, P, F, P).transpose(2, 1, 0, 3)
        ).astype(BF16)
        in1.append({"XT0": XT0, "XT": XT_T, "W": W_T, "BE": be_f[e]})
    return in1, Bcap


def prep_l2_inputs(xf, d, H, Wo, bo, norm_w):
    """Per-core L2 inputs. CT = g1*A + g2*B combined on host (fp32 math,
    one bf16 downcast); XIN = x + bo in fp32."""
    T, D = xf.shape
    TPC = T // NCORE
    KB = D // P // 4
    NC4 = D // 512
    # pretile Wo into contiguous (k-bundle, n-chunk) blocks
    Wo_b = np.ascontiguousarray(
        np.asarray(Wo, np.float32)
        .reshape(KB, 4, P, NC4, 512)
        .transpose(0, 3, 2, 1, 4)
    ).astype(BF16)
    bo_f = np.asarray(bo, np.float32)
    nw_f = np.asarray(norm_w, np.float32)
    e1, e2, pos = d["e1"], d["e2"], d["pos"]
    g1, g2 = d["g1"], d["g2"]
    in2 = []
    for c in range(NCORE):
        tl = np.arange(c * TPC, (c + 1) * TPC)
        CTf = np.empty((D, TPC), np.float32)
        BTf = np.empty((D, TPC), np.float32)
        for e in range(H.shape[0]):
            s1 = e1[tl] == e
            if s1.any():
                CTf[:, s1] = H[e][:, pos[tl[s1], 0]]
            s2 = e2[tl] == e
            if s2.any():
                BTf[:, s2] = H[e][:, pos[tl[s2], 1]]
        CTf = CTf * g1[tl][None, :] + BTf * g2[tl][None, :]
        CTt = np.ascontiguousarray(
            CTf.reshape(KB, 4, P, TPC).transpose(0, 2, 1, 3)
        ).astype(BF16)
        XIN = (xf[tl] + bo_f[None, :]).astype(BF16)
        in2.append({"CT": CTt, "XIN": XIN, "WO": Wo_b, "NW": nw_f})
    return in2


# ----------------------------------------------------------------------------
# Harness entry point: full (unsharded) inputs -> full output.
# ----------------------------------------------------------------------------
_L1_CACHE = {}
_L2_CACHE = {}


def kernel(x, Wr, br, We, be, Wo, bo, norm_w):
    B, S, D = x.shape
    E = We.shape[0]
    T = B * S
    TPC = T // NCORE
    xf = np.ascontiguousarray(np.asarray(x, np.float32).reshape(T, D))
    d = host_dispatch(xf, np.asarray(Wr, np.float32), np.asarray(br, np.float32))

    in1, Bcap = prep_l1_inputs(xf, d, We, be)
    if (D, Bcap) not in _L1_CACHE:
        _L1_CACHE[(D, Bcap)] = build_l1(D, Bcap)
    r1 = run_bass_kernel_spmd(_L1_CACHE[(D, Bcap)], in1, list(range(NCORE)))
    H = np.stack([r1.results[e]["H"] for e in range(E)])

    in2 = prep_l2_inputs(xf, d, H, Wo, bo, norm_w)
    unit_nw = bool(np.all(np.asarray(norm_w, np.float32) == 1.0))
    if (D, TPC, unit_nw) not in _L2_CACHE:
        _L2_CACHE[(D, TPC, unit_nw)] = build_l2(D, TPC, unit_nw)
    r2 = run_bass_kernel_spmd(_L2_CACHE[(D, TPC, unit_nw)], in2, list(range(NCORE)))
    Y = np.concatenate([r2.results[c]["Y"] for c in range(NCORE)], axis=0)
    return Y.reshape(B, S, D).astype(np.asarray(x).dtype)
